# revision 67
# baseline (speedup 1.0000x reference)
"""Distributed Trainium2 Bass kernel for the CARAFE+SWT+CSPRep block.

Strategy: pure data parallel — 16 samples sharded 2-per-core across 8
NeuronCores; all weights replicated. The whole per-sample network runs
on-chip in one NEFF:

  hr 1x1 conv (f32r) -> ce 3x3 conv (bf16) -> exp -> carafe (window DMA
  + partition-sum matmuls + reciprocal) -> wh_in -> [SWT+concat+1x1
  folded into a 2x2 stencil conv] c1/c2 -> 3x RepVgg (3x3+1x1, bf16)
  -> sup 3x3 -> +feat residual -> pos 1x1 (f32r).

Key rewrites (validated against the reference in numpy):
  * The stationary Haar bands (lh,hl,hh) are linear in 2x2 shifted
    copies of wh_in, so conv1x1(concat(bands)) == conv2x2(wh_in) with
    host-transformed weights (clamped edge handled by a 97x97 padded
    buffer with replicated top row / left col).
  * CARAFE softmax-normalize + weighted sum == (sum_k e_k*v_k)/(sum_k
    e_k) with e = exp(raw); the k=25 shifted upsampled views of pred
    are materialized by a single strided-window DMA from a host-built
    reflect-padded 2x-upsampled pred plane (pure indexing, no math).
  * BN scale/shift fused into the PSUM->SBUF evacuation on ScalarE
    (out = Relu(psum*scale + bias)); relu(relu(a)+relu(b)) == the sum.
"""

import numpy as np
import ml_dtypes

import bass_rust
import concourse.bass as bass
import concourse.tile as tile
from concourse import bacc, mybir
from concourse.bass_utils import run_bass_kernel_spmd

F32 = mybir.dt.float32
F32R = mybir.dt.float32r
BF16 = mybir.dt.bfloat16
AFT = mybir.ActivationFunctionType

NCORES = 8
SPC = 2            # samples per core
PIX = 96 * 96      # 9216
GR = 4             # output rows per matmul tile
G = 96 // GR       # 24 row groups
NT = GR * 96       # 384 free elems per matmul
DELTAS = [(0, 0), (0, 1), (1, 0), (1, 1)]

_CACHE = {}
DEBUG_TAPS = False  # emit extra dram outputs of intermediates
MAX_PHASE = 99      # debug: limit emission (4=c1, 5=rep0, 6=rep1, 7=rep2, 8=c2, 9=sup)
REC_OFF = False     # debug: replace reciprocal with copy
SKIP_OPS = ()       # debug: skip named ops ("exp","vdma","tg","sums","pub","whin","uedge")


def _r3(ap, a=GR):
    """[P, a*b] -> [P, a, b] view."""
    return ap.rearrange("p (a b) -> p a b", a=a)


def _win_ap(p2s_ap, s, g, ki):
    """Strided window view of the upsampled padded pred plane: CARAFE
    taps (ki, 0..4) for output rows [4g, 4g+4) as a [5,4,96] DMA source."""
    w = p2s_ap.copy()
    w.ap = bass_rust.VecI64Pair([[2, 5], [104, GR], [1, 96]])
    w.offset = s * 104 * 104 + GR * g * 104 + 2 * 104 * ki
    return w


def _build_program():
    nc = bacc.Bacc(
        "TRN2",
        target_bir_lowering=False,
        debug=False,
        enable_asserts=False,
        num_devices=NCORES,
    )

    def din(name, shape, dt=F32):
        return nc.dram_tensor(name, shape, dt, kind="ExternalInput").ap()

    def dout(name, shape, dt=F32):
        return nc.dram_tensor(name, shape, dt, kind="ExternalOutput").ap()

    feat = din("feat", [SPC, 2, 128, PIX])
    featb = din("featb", [SPC, 2, 128, PIX], BF16)
    p2s = din("p2s", [SPC, 104, 104])
    hrw = din("hrw", [2, 128, 64], BF16)
    hrb = din("hrb", [64, 1])
    cew = din("cew", [64, 9, 25], BF16)
    ceb = din("ceb", [25, 1])
    cw = din("cw", [2, 2, 128, 4, 256], BF16)    # conv, ci_t, ci, delta, co
    cs = din("cs", [2, 2, 128, 1])               # conv, co_t, co, 1
    cb = din("cb", [2, 2, 128, 1])
    rw3 = din("rw3", [3, 2, 128, 9, 256], BF16)
    rs3 = din("rs3", [3, 2, 128, 1])
    rb3 = din("rb3", [3, 2, 128, 1])
    rw1 = din("rw1", [3, 2, 128, 256], BF16)
    rs1 = din("rs1", [3, 2, 128, 1])
    rb1 = din("rb1", [3, 2, 128, 1])
    sw = din("sw", [2, 128, 9, 256], BF16)
    ss = din("ss", [2, 128, 1])
    sbe = din("sbe", [2, 128, 1])
    pw = din("pw", [2, 128, 1])
    pb = din("pb", [1, 1])

    o_d = dout("o", [SPC, 2, 128, PIX])
    pos_d = dout("pos", [SPC, PIX])
    pup_d = dout("pup", [SPC, PIX], F32R)
    dbg = None
    if DEBUG_TAPS:
        dbg = {k: dout(f"dbg_{k}", [SPC, 2, 128, 98 * 98], BF16)
               for k in ("u", "x1", "xr", "wh")}
        dbg["wc1"] = dout("dbg_wc1", [SPC, 2, 128, 9 * 256], BF16)
        dbg["wc1pre"] = dout("dbg_wc1pre", [SPC, 2, 128, 9 * 256], BF16)
        dbg["guard"] = dout("dbg_guard", [SPC, 128, 16], F32)
        dbg["gpad"] = dout("dbg_gpad", [SPC, 64, 98 * 98], BF16)
        dbg["v"] = dout("dbg_v", [SPC, 25, PIX], F32)
        dbg["e"] = dout("dbg_e", [SPC, 25, PIX], F32R)

    with tile.TileContext(nc) as tc:
        with (
            tc.tile_pool(name="wconst", bufs=1) as wp,
            tc.tile_pool(name="wbig", bufs=4) as wbp,
            tc.tile_pool(name="pad", bufs=4) as padp,
            tc.tile_pool(name="upool", bufs=2) as upool,
            tc.tile_pool(name="rot", bufs=3) as rot,
            tc.tile_pool(name="psp", bufs=4, space="PSUM") as psp,
        ):
            # ---- persistent small weights ----
            Whr = []
            for t in range(2):
                w = wp.tile([128, 64], BF16, name=f"whr{t}", tag=f"whr{t}")
                nc.sync.dma_start(w[:], hrw[t])
                Whr.append(w)
            Wce = wp.tile([64, 9, 25], BF16, name="wce", tag="wce")
            nc.sync.dma_start(Wce[:], cew[:])
            Wr1 = [[None] * 2 for _ in range(3)]
            for i in range(3):
                for t in range(2):
                    w = wp.tile([128, 256], BF16, name=f"wr1_{i}{t}", tag=f"wr1_{i}{t}")
                    nc.sync.dma_start(w[:], rw1[i, t])
                    Wr1[i][t] = w
            Wc = [[None] * 2 for _ in range(2)]
            for cv in range(2):
                for t in range(2):
                    w = wp.tile([128, 4, 256], BF16, name=f"wcv{cv}{t}", tag=f"wcv{cv}{t}")
                    nc.sync.dma_start(w[:], cw[cv, t])
                    Wc[cv][t] = w
            Wr3 = [[None] * 2 for _ in range(3)]
            for i in range(3):
                for t in range(2):
                    w = wp.tile([128, 9, 256], BF16, name=f"wr3_{i}{t}", tag=f"wr3_{i}{t}")
                    nc.sync.dma_start(w[:], rw3[i, t])
                    Wr3[i][t] = w
            Wsup = []
            for t in range(2):
                w = wp.tile([128, 9, 256], BF16, name=f"wsup{t}", tag=f"wsup{t}")
                nc.sync.dma_start(w[:], sw[t])
                Wsup.append(w)
            Wpos = []
            for t in range(2):
                w = wp.tile([128, 1], F32, name=f"wpos{t}", tag=f"wpos{t}")
                nc.sync.dma_start(w[:], pw[t])
                Wpos.append(w)

            def vec(name, src, p=128):
                v = wp.tile([p, 1], F32, name=name, tag=name)
                nc.sync.dma_start(v[:], src)
                return v

            Bhr = vec("bhr", hrb[:], p=64)
            Bce = vec("bce", ceb[:], p=25)
            Bpos = vec("bpos", pb[:], p=1)
            Sc = [[vec(f"sc{c}{t}", cs[c, t]) for t in range(2)] for c in range(2)]
            Bc = [[vec(f"bc{c}{t}", cb[c, t]) for t in range(2)] for c in range(2)]
            Sr3 = [[vec(f"sr3_{i}{t}", rs3[i, t]) for t in range(2)] for i in range(3)]
            Br3 = [[vec(f"br3_{i}{t}", rb3[i, t]) for t in range(2)] for i in range(3)]
            Sr1 = [[vec(f"sr1_{i}{t}", rs1[i, t]) for t in range(2)] for i in range(3)]
            Br1 = [[vec(f"br1_{i}{t}", rb1[i, t]) for t in range(2)] for i in range(3)]
            Ssup = [vec(f"ssup{t}", ss[t]) for t in range(2)]
            Bsup = [vec(f"bsup{t}", sbe[t]) for t in range(2)]

            # f32r constants must come from a rounding producer (DVE copy)
            ones_f = wp.tile([25, 128], F32, name="ones_f", tag="ones_f")
            nc.vector.memset(ones_f[:], 1.0)
            ones25 = wp.tile([25, 1], F32R, name="ones25", tag="ones25")
            nc.vector.tensor_copy(ones25[:], ones_f[:, 0:1])
            ones128 = wp.tile([1, 128], F32R, name="ones128", tag="ones128")
            nc.vector.tensor_copy(ones128[:], ones_f[0:1, :])
            guard = None
            if DEBUG_TAPS:
                guard = wp.tile([128, 16], F32, name="guard", tag="guard")
                nc.vector.memset(guard[:], 0.0)

            for s in range(SPC):
                _emit_sample(
                    nc, tc, s,
                    feat=feat, p2s=p2s,
                    o_d=o_d, pos_d=pos_d, pup_d=pup_d,
                    Whr=Whr, Wce=Wce, Wr1=Wr1, Wpos=Wpos,
                    Wc=Wc, Wr3=Wr3, Wsup=Wsup,
                    Bhr=Bhr, Bce=Bce, Bpos=Bpos, Sc=Sc, Bc=Bc,
                    Sr3=Sr3, Br3=Br3, Sr1=Sr1, Br1=Br1,
                    Ssup=Ssup, Bsup=Bsup, ones25=ones25, ones128=ones128,
                    featb=featb,
                    wbp=wbp, padp=padp, upool=upool, rot=rot, psp=psp,
                    dbg=dbg, guard=guard,
                )

    nc.compile()
    return nc


def _emit_sample(nc, tc, s, *, feat, p2s, o_d, pos_d, pup_d,
                 Whr, Wce, Wr1, Wpos, Wc, Wr3, Wsup, Bhr, Bce, Bpos, Sc, Bc,
                 Sr3, Br3, Sr1, Br1, Ssup, Bsup, ones25, ones128, featb,
                 wbp, padp, upool, rot, psp, dbg=None, guard=None):

    def dump(key, tiles, pad=98):
        if dbg is None:
            return
        for t in range(2):
            n = pad * pad
            nc.sync.dma_start(dbg[key][s, t, :, 0:n],
                              tiles[t][:, 0:pad, 0:pad])

    def pad_tile(name, p=128):
        t = padp.tile([p, 98, 98], BF16, name=f"{name}_s{s}", tag="pad")
        return t

    def zero_borders(t):
        nc.gpsimd.memset(t[:, 0, :], 0.0)
        nc.gpsimd.memset(t[:, 97, :], 0.0)
        nc.gpsimd.memset(t[:, 1:97, 0], 0.0)
        nc.gpsimd.memset(t[:, 1:97, 97], 0.0)

    gsl = lambda g: slice(g * NT, (g + 1) * NT)

    # ---------- Phase A: hr 1x1 conv (f32r) -> padded guide ----------
    Gpad = pad_tile("gpad", p=64)
    zero_borders(Gpad)
    for g in range(G):
        ps = psp.tile([64, GR, 96], F32, name="ps_hr", tag="ps")
        for t in range(2):
            fs = rot.tile([128, NT], BF16, name="fsl", tag="fsl", bufs=5)
            nc.sync.dma_start(fs[:], featb[s, t, :, gsl(g)])
            nc.tensor.matmul(
                ps[:], Whr[t][:], _r3(fs[:]),
                start=(t == 0), stop=(t == 1),
            )
        nc.scalar.activation(
            Gpad[:, 1 + GR * g:1 + GR * (g + 1), 1:97], ps[:],
            AFT.Identity, bias=Bhr[:],
        )

    if dbg is not None:
        nc.sync.dma_start(dbg["gpad"][s], Gpad[:])

    # ---------- Phase B+C: ce 3x3 conv -> exp -> carafe -> wh_in/U ----------
    U = [upool.tile([128, 97, 97], BF16, name=f"u{t}_s{s}", tag="u") for t in range(2)]
    for g in range(G):
        psE = psp.tile([25, GR, 96], F32, name="ps_ce", tag="ps")
        for o9 in range(9):
            ky, kx = divmod(o9, 3)
            nc.tensor.matmul(
                psE[:], Wce[:, o9, :],
                Gpad[:, GR * g + ky:GR * g + ky + GR, kx:kx + 96],
                start=(o9 == 0), stop=(o9 == 8),
            )
        eg = rot.tile([25, GR, 96], F32R, name="eg", tag="eg", bufs=3)
        if "exp" in SKIP_OPS:
            nc.vector.tensor_copy(eg[:], psE[:])
        else:
            nc.scalar.activation(eg[:], psE[:], AFT.Exp, bias=Bce[:])
        vg = rot.tile([25, GR, 96], F32, name="vg", tag="vg", bufs=3)
        if "vdma" not in SKIP_OPS:
            for ki in range(5):
                nc.sync.dma_start(vg[5 * ki:5 * ki + 5], _win_ap(p2s, s, g, ki))
        else:
            nc.vector.memset(vg[:], 1.0)
        if dbg is not None:
            nc.sync.dma_start(dbg["v"][s, :, gsl(g)], vg[:])
            nc.sync.dma_start(dbg["e"][s, :, gsl(g)], eg[:])
        tg = rot.tile([25, GR, 96], F32R, name="tg", tag="tg", bufs=3)
        if "tg" in SKIP_OPS:
            nc.vector.tensor_copy(tg[:], vg[:])
        else:
            nc.vector.tensor_mul(tg[:], eg[:], vg[:])
        psN = psp.tile([1, GR, 96], F32, name="ps_n", tag="psn", bufs=4)
        nc.tensor.matmul(psN[:], ones25[:], tg[:], start=True, stop=True)
        psD = psp.tile([1, GR, 96], F32, name="ps_d", tag="psn", bufs=4)
        nc.tensor.matmul(psD[:], ones25[:], eg[:], start=True, stop=True)
        rc = rot.tile([1, GR, 96], F32, name="rc", tag="sm", bufs=6)
        if REC_OFF:
            nc.vector.tensor_copy(rc[:], psD[:])
        else:
            nc.vector.reciprocal(rc[:], psD[:])
        pu = rot.tile([1, GR, 96], F32R, name="pu", tag="sm", bufs=6)
        nc.vector.tensor_mul(pu[:], psN[:], rc[:])
        nc.gpsimd.dma_start(pup_d[s, gsl(g)], pu[:])
        pub = psp.tile([128, GR, 96], F32, name="pub", tag="ps")
        if "pub" in SKIP_OPS:
            nc.vector.memset(pub[:], 1.0)
        else:
            nc.tensor.matmul(pub[:], ones128[:], pu[:], start=True, stop=True)
        for t in range(2):
            fs = rot.tile([128, NT], F32, name="fsl2", tag="fsl", bufs=5)
            nc.sync.dma_start(fs[:], feat[s, t, :, gsl(g)])
            if "whin" in SKIP_OPS:
                nc.vector.tensor_copy(
                    U[t][:, 1 + GR * g:1 + GR * (g + 1), 1:97], _r3(fs[:]))
            else:
                nc.vector.tensor_mul(
                    U[t][:, 1 + GR * g:1 + GR * (g + 1), 1:97],
                    _r3(fs[:]), pub[:],
                )
    for t in range(2):
        if "uedge" in SKIP_OPS:
            continue
        nc.vector.tensor_copy(U[t][:, 0, 1:97], U[t][:, 1, 1:97])
        nc.vector.tensor_copy(U[t][:, :, 0], U[t][:, :, 1])
    dump("u", U, pad=97)

    # ---------- stencil conv helper (c1 / c2) ----------
    def cconv_mm(Wt, co_t, g, ps):
        k = 0
        for t in range(2):
            for d, (dy, dx) in enumerate(DELTAS):
                nc.tensor.matmul(
                    ps[:], Wt[t][:, d, co_t * 128:(co_t + 1) * 128],
                    U[t][:, GR * g + dy:GR * g + dy + GR, dx:dx + 96],
                    start=(k == 0), stop=(k == 7),
                )
                k += 1

    # ---------- Phase D: c1 -> P0 ----------
    Wc1 = Wc[0]
    if dbg is not None:
        for t in range(2):
            nc.sync.dma_start(dbg["wc1pre"][s, t, :, 0:4 * 256], Wc1[t][:])
    P0 = [pad_tile(f"p0_{t}") for t in range(2)]
    for t in range(2):
        zero_borders(P0[t])
    for co_t in range(2):
        for g in range(G):
            ps = psp.tile([128, GR, 96], F32, name="ps_c1", tag="ps")
            cconv_mm(Wc1, co_t, g, ps)
            nc.scalar.activation(
                P0[co_t][:, 1 + GR * g:1 + GR * (g + 1), 1:97], ps[:],
                AFT.Relu, bias=Bc[0][co_t][:], scale=Sc[0][co_t][:],
            )

    if dbg is not None:
        for t in range(2):
            nc.sync.dma_start(dbg["wc1"][s, t, :, 0:4 * 256], Wc1[t][:])
        nc.sync.dma_start(dbg["guard"][s], guard[:])
    dump("x1", P0)
    if MAX_PHASE <= 4:
        return

    # ---------- Phase E: RepVgg blocks ----------
    cur = P0
    for i in range(3):
        if MAX_PHASE <= 4 + i:
            break
        W3 = Wr3[i]
        nxt = [pad_tile(f"p{i + 1}_{t}") for t in range(2)]
        for t in range(2):
            zero_borders(nxt[t])
        for co_t in range(2):
            co = slice(co_t * 128, (co_t + 1) * 128)
            for g in range(G):
                ps3 = psp.tile([128, GR, 96], F32, name="ps_r3", tag="ps")
                k = 0
                for t in range(2):
                    for o9 in range(9):
                        ky, kx = divmod(o9, 3)
                        nc.tensor.matmul(
                            ps3[:], W3[t][:, o9, co],
                            cur[t][:, GR * g + ky:GR * g + ky + GR, kx:kx + 96],
                            start=(k == 0), stop=(k == 17),
                        )
                        k += 1
                ps1 = psp.tile([128, GR, 96], F32, name="ps_r1", tag="ps")
                for t in range(2):
                    nc.tensor.matmul(
                        ps1[:], Wr1[i][t][:, co],
                        cur[t][:, 1 + GR * g:1 + GR * (g + 1), 1:97],
                        start=(t == 0), stop=(t == 1),
                    )
                b3 = rot.tile([128, GR, 96], F32, name="b3", tag="b3")
                nc.scalar.activation(b3[:], ps3[:], AFT.Relu,
                                     bias=Br3[i][co_t][:], scale=Sr3[i][co_t][:])
                b1 = rot.tile([128, GR, 96], F32, name="b1", tag="b1")
                nc.scalar.activation(b1[:], ps1[:], AFT.Relu,
                                     bias=Br1[i][co_t][:], scale=Sr1[i][co_t][:])
                nc.vector.tensor_add(
                    nxt[co_t][:, 1 + GR * g:1 + GR * (g + 1), 1:97], b3[:], b1[:])
        cur = nxt
    dump("xr", cur)
    if MAX_PHASE <= 7:
        return

    # ---------- Phase F: c2 + merge -> WH ----------
    Wc2 = Wc[1]
    WH = [pad_tile(f"wh_{t}") for t in range(2)]
    for t in range(2):
        zero_borders(WH[t])
    for co_t in range(2):
        for g in range(G):
            ps = psp.tile([128, GR, 96], F32, name="ps_c2", tag="ps")
            cconv_mm(Wc2, co_t, g, ps)
            x2t = rot.tile([128, GR, 96], F32, name="x2t", tag="b3")
            nc.scalar.activation(x2t[:], ps[:], AFT.Relu,
                                 bias=Bc[1][co_t][:], scale=Sc[1][co_t][:])
            t1 = rot.tile([128, GR, 96], F32, name="t1", tag="b1")
            nc.vector.tensor_add(
                t1[:], x2t[:], cur[co_t][:, 1 + GR * g:1 + GR * (g + 1), 1:97])
            nc.vector.tensor_add(
                WH[co_t][:, 1 + GR * g:1 + GR * (g + 1), 1:97], t1[:],
                U[co_t][:, 1 + GR * g:1 + GR * (g + 1), 1:97])

    dump("wh", WH)
    if MAX_PHASE <= 8:
        return

    # ---------- Phase G+H: sup 3x3 + residual + pos 1x1 ----------
    for g in range(G):
        ots = []
        for co_t in range(2):
            co = slice(co_t * 128, (co_t + 1) * 128)
            ps = psp.tile([128, GR, 96], F32, name="ps_sup", tag="ps")
            k = 0
            for t in range(2):
                for o9 in range(9):
                    ky, kx = divmod(o9, 3)
                    nc.tensor.matmul(
                        ps[:], Wsup[t][:, o9, co],
                        WH[t][:, GR * g + ky:GR * g + ky + GR, kx:kx + 96],
                        start=(k == 0), stop=(k == 17),
                    )
                    k += 1
            st = rot.tile([128, GR, 96], F32, name="st", tag="b3")
            nc.scalar.activation(st[:], ps[:], AFT.Relu,
                                 bias=Bsup[co_t][:], scale=Ssup[co_t][:])
            fs = rot.tile([128, NT], F32, name="fsl3", tag="fsl", bufs=5)
            nc.sync.dma_start(fs[:], feat[s, co_t, :, gsl(g)])
            ot = rot.tile([128, GR, 96], F32, name="ot", tag="ot", bufs=3)
            nc.vector.tensor_add(ot[:], st[:], _r3(fs[:]))
            nc.gpsimd.dma_start(o_d[s, co_t, :, gsl(g)], ot[:])
            ots.append(ot)
        psP = psp.tile([1, GR, 96], F32, name="ps_pos", tag="psn", bufs=4)
        for co_t in range(2):
            nc.tensor.matmul(psP[:], Wpos[co_t][:], ots[co_t][:],
                             start=(co_t == 0), stop=(co_t == 1))
        pt = rot.tile([1, GR, 96], F32, name="pt", tag="sm", bufs=6)
        nc.scalar.activation(pt[:], psP[:], AFT.Identity, bias=Bpos[:])
        nc.gpsimd.dma_start(pos_d[s, gsl(g)], pt[:])


# ============================================================
# host side
# ============================================================

def _prep_weights(inputs):
    bf = ml_dtypes.bfloat16
    f = lambda x: np.ascontiguousarray(np.asarray(x, np.float32))
    w = {}
    w["hrw"] = np.ascontiguousarray(
        np.asarray(inputs["hrW"], np.float32)[:, :, 0, 0].T.reshape(2, 128, 64).astype(bf))
    w["hrb"] = f(inputs["hrB"]).reshape(64, 1)
    w["cew"] = np.ascontiguousarray(
        np.asarray(inputs["ceW"], np.float32).transpose(1, 2, 3, 0).reshape(64, 9, 25).astype(bf))
    w["ceb"] = f(inputs["ceB"]).reshape(25, 1)

    s = np.float32(0.5)
    cws = []
    for key in ("c1W", "c2W"):
        cW = f(inputs[key])[:, :, 0, 0]  # (256, 768)
        W_lh, W_hl, W_hh = cW[:, :256], cW[:, 256:512], cW[:, 512:768]
        Wd = np.stack([
            s * (W_lh + W_hl + W_hh),
            s * (-W_lh + W_hl - W_hh),
            s * (W_lh - W_hl - W_hh),
            s * (-W_lh - W_hl + W_hh),
        ])  # (4, co, ci)
        # -> [ci_t, 128ci, 4d, 256co]
        cws.append(Wd.transpose(2, 0, 1).reshape(2, 128, 4, 256))
    w["cw"] = np.ascontiguousarray(np.stack(cws).astype(bf))
    w["cs"] = np.stack([f(inputs["c1s"]), f(inputs["c2s"])]).reshape(2, 2, 128, 1)
    w["cb"] = np.stack([f(inputs["c1b"]), f(inputs["c2b"])]).reshape(2, 2, 128, 1)

    # repW3: (3, co, ci, ky, kx) -> [i, ci_t, 128ci, 9off, 256co]
    r3 = f(inputs["repW3"]).transpose(0, 2, 3, 4, 1).reshape(3, 2, 128, 9, 256)
    w["rw3"] = np.ascontiguousarray(r3.astype(bf))
    w["rs3"] = f(inputs["repS3"]).reshape(3, 2, 128, 1)
    w["rb3"] = f(inputs["repB3"]).reshape(3, 2, 128, 1)
    r1 = f(inputs["repW1"])[:, :, :, 0, 0].transpose(0, 2, 1).reshape(3, 2, 128, 256)
    w["rw1"] = np.ascontiguousarray(r1.astype(bf))
    w["rs1"] = f(inputs["repS1"]).reshape(3, 2, 128, 1)
    w["rb1"] = f(inputs["repB1"]).reshape(3, 2, 128, 1)

    sW = f(inputs["supW"]).transpose(1, 2, 3, 0).reshape(2, 128, 9, 256)
    w["sw"] = np.ascontiguousarray(sW.astype(bf))
    w["ss"] = f(inputs["supS"]).reshape(2, 128, 1)
    w["sbe"] = (f(inputs["supCb"]) * f(inputs["supS"]) + f(inputs["supB"])).reshape(2, 128, 1)

    w["pw"] = f(inputs["posW"])[0, :, 0, 0].reshape(2, 128, 1)
    w["pb"] = f(inputs["posB"]).reshape(1, 1)
    return w


def kernel(**inputs):
    nc = _CACHE.get("nc")
    if nc is None:
        nc = _build_program()
        _CACHE["nc"] = nc

    feat = np.ascontiguousarray(np.asarray(inputs["feat"], np.float32))
    featb = np.ascontiguousarray(feat.astype(ml_dtypes.bfloat16))
    pred = np.asarray(inputs["pred"], np.float32)
    B = feat.shape[0]

    xp = np.pad(pred[:, 0], ((0, 0), (2, 2), (2, 2)), mode="reflect")
    p2 = np.ascontiguousarray(xp.repeat(2, axis=1).repeat(2, axis=2))  # (B,104,104)

    w = _prep_weights(inputs)
    in_maps = []
    for c in range(NCORES):
        m = dict(w)
        m["feat"] = feat[SPC * c:SPC * (c + 1)].reshape(SPC, 2, 128, PIX)
        m["featb"] = featb[SPC * c:SPC * (c + 1)].reshape(SPC, 2, 128, PIX)
        m["p2s"] = p2[SPC * c:SPC * (c + 1)]
        in_maps.append(m)

    res = run_bass_kernel_spmd(nc, in_maps, core_ids=list(range(NCORES)))
    outs = res.results

    out = np.concatenate([r["o"].reshape(SPC, 256, 96, 96) for r in outs])
    pos = np.concatenate([r["pos"].reshape(SPC, 1, 96, 96) for r in outs])
    pup = np.concatenate([r["pup"].reshape(SPC, 1, 96, 96) for r in outs])
    return out, pos, pup


# revision 72
# speedup vs baseline: 1.0105x; 1.0105x over previous
"""Distributed Trainium2 Bass kernel for the CARAFE+SWT+CSPRep block.

Strategy: pure data parallel — 16 samples sharded 2-per-core across 8
NeuronCores; all weights replicated. The whole per-sample network runs
on-chip in one NEFF:

  hr 1x1 conv (f32r) -> ce 3x3 conv (bf16) -> exp -> carafe (window DMA
  + partition-sum matmuls + reciprocal) -> wh_in -> [SWT+concat+1x1
  folded into a 2x2 stencil conv] c1/c2 -> 3x RepVgg (3x3+1x1, bf16)
  -> sup 3x3 -> +feat residual -> pos 1x1 (f32r).

Key rewrites (validated against the reference in numpy):
  * The stationary Haar bands (lh,hl,hh) are linear in 2x2 shifted
    copies of wh_in, so conv1x1(concat(bands)) == conv2x2(wh_in) with
    host-transformed weights (clamped edge handled by a 97x97 padded
    buffer with replicated top row / left col).
  * CARAFE softmax-normalize + weighted sum == (sum_k e_k*v_k)/(sum_k
    e_k) with e = exp(raw); the k=25 shifted upsampled views of pred
    are materialized by a single strided-window DMA from a host-built
    reflect-padded 2x-upsampled pred plane (pure indexing, no math).
  * BN scale/shift fused into the PSUM->SBUF evacuation on ScalarE
    (out = Relu(psum*scale + bias)); relu(relu(a)+relu(b)) == the sum.
"""

import numpy as np
import ml_dtypes

import bass_rust
import concourse.bass as bass
import concourse.tile as tile
from concourse import bacc, mybir
from concourse.bass_utils import run_bass_kernel_spmd

F32 = mybir.dt.float32
F32R = mybir.dt.float32r
BF16 = mybir.dt.bfloat16
AFT = mybir.ActivationFunctionType

NCORES = 8
SPC = 2            # samples per core
PIX = 96 * 96      # 9216
GR = 4             # output rows per matmul tile
G = 96 // GR       # 24 row groups
NT = GR * 96       # 384 free elems per matmul
DELTAS = [(0, 0), (0, 1), (1, 0), (1, 1)]

_CACHE = {}
DEBUG_TAPS = False  # emit extra dram outputs of intermediates
MAX_PHASE = 99      # debug: limit emission (4=c1, 5=rep0, 6=rep1, 7=rep2, 8=c2, 9=sup)
REC_OFF = False     # debug: replace reciprocal with copy
SKIP_OPS = ()       # debug: skip named ops ("exp","vdma","tg","sums","pub","whin","uedge")


def _r3(ap, a=GR):
    """[P, a*b] -> [P, a, b] view."""
    return ap.rearrange("p (a b) -> p a b", a=a)


def _win_ap(p2s_ap, s, g, ki):
    """Strided window view of the upsampled padded pred plane: CARAFE
    taps (ki, 0..4) for output rows [4g, 4g+4) as a [5,4,96] DMA source."""
    w = p2s_ap.copy()
    w.ap = bass_rust.VecI64Pair([[2, 5], [104, GR], [1, 96]])
    w.offset = s * 104 * 104 + GR * g * 104 + 2 * 104 * ki
    return w


def _build_program():
    nc = bacc.Bacc(
        "TRN2",
        target_bir_lowering=False,
        debug=False,
        enable_asserts=False,
        num_devices=NCORES,
    )

    def din(name, shape, dt=F32):
        return nc.dram_tensor(name, shape, dt, kind="ExternalInput").ap()

    def dout(name, shape, dt=F32):
        return nc.dram_tensor(name, shape, dt, kind="ExternalOutput").ap()

    feat = din("feat", [SPC, 2, 128, PIX])
    featb = din("featb", [SPC, 2, 128, PIX], BF16)
    p2s = din("p2s", [SPC, 104, 104])
    hrw = din("hrw", [2, 128, 64], BF16)
    hrb = din("hrb", [64, 1])
    cew = din("cew", [64, 9, 25], BF16)
    ceb = din("ceb", [25, 1])
    cw = din("cw", [2, 2, 128, 4, 256], BF16)    # conv, ci_t, ci, delta, co
    cs = din("cs", [2, 2, 128, 1])               # conv, co_t, co, 1
    cb = din("cb", [2, 2, 128, 1])
    rw3 = din("rw3", [3, 2, 128, 9, 256], BF16)
    rs3 = din("rs3", [3, 2, 128, 1])
    rb3 = din("rb3", [3, 2, 128, 1])
    rw1 = din("rw1", [3, 2, 128, 256], BF16)
    rs1 = din("rs1", [3, 2, 128, 1])
    rb1 = din("rb1", [3, 2, 128, 1])
    sw = din("sw", [2, 128, 9, 256], BF16)
    ss = din("ss", [2, 128, 1])
    sbe = din("sbe", [2, 128, 1])
    pw = din("pw", [2, 128, 1])
    pb = din("pb", [1, 1])

    o_d = dout("o", [SPC, 2, 128, PIX])
    pos_d = dout("pos", [SPC, PIX])
    pup_d = dout("pup", [SPC, PIX], F32R)
    dbg = None
    if DEBUG_TAPS:
        dbg = {k: dout(f"dbg_{k}", [SPC, 2, 128, 98 * 98], BF16)
               for k in ("u", "x1", "xr", "wh")}
        dbg["wc1"] = dout("dbg_wc1", [SPC, 2, 128, 9 * 256], BF16)
        dbg["wc1pre"] = dout("dbg_wc1pre", [SPC, 2, 128, 9 * 256], BF16)
        dbg["guard"] = dout("dbg_guard", [SPC, 128, 16], F32)
        dbg["gpad"] = dout("dbg_gpad", [SPC, 64, 98 * 98], BF16)
        dbg["v"] = dout("dbg_v", [SPC, 25, PIX], F32)
        dbg["e"] = dout("dbg_e", [SPC, 25, PIX], F32R)

    with tile.TileContext(nc) as tc:
        with (
            tc.tile_pool(name="wconst", bufs=1) as wp,
            tc.tile_pool(name="wbig", bufs=4) as wbp,
            tc.tile_pool(name="pad", bufs=4) as padp,
            tc.tile_pool(name="upool", bufs=2) as upool,
            tc.tile_pool(name="rot", bufs=3) as rot,
            tc.tile_pool(name="psp", bufs=6, space="PSUM") as psp,
        ):
            # ---- persistent small weights ----
            Whr = []
            for t in range(2):
                w = wp.tile([128, 64], BF16, name=f"whr{t}", tag=f"whr{t}")
                nc.sync.dma_start(w[:], hrw[t])
                Whr.append(w)
            Wce = wp.tile([64, 9, 25], BF16, name="wce", tag="wce")
            nc.sync.dma_start(Wce[:], cew[:])
            Wr1 = [[None] * 2 for _ in range(3)]
            for i in range(3):
                for t in range(2):
                    w = wp.tile([128, 256], BF16, name=f"wr1_{i}{t}", tag=f"wr1_{i}{t}")
                    nc.sync.dma_start(w[:], rw1[i, t])
                    Wr1[i][t] = w
            Wc = [[None] * 2 for _ in range(2)]
            for cv in range(2):
                for t in range(2):
                    w = wp.tile([128, 4, 256], BF16, name=f"wcv{cv}{t}", tag=f"wcv{cv}{t}")
                    nc.sync.dma_start(w[:], cw[cv, t])
                    Wc[cv][t] = w
            Wr3 = [[None] * 2 for _ in range(3)]
            for i in range(3):
                for t in range(2):
                    w = wp.tile([128, 9, 256], BF16, name=f"wr3_{i}{t}", tag=f"wr3_{i}{t}")
                    nc.scalar.dma_start(w[:], rw3[i, t])
                    Wr3[i][t] = w
            Wsup = []
            for t in range(2):
                w = wp.tile([128, 9, 256], BF16, name=f"wsup{t}", tag=f"wsup{t}")
                nc.scalar.dma_start(w[:], sw[t])
                Wsup.append(w)
            Wpos = []
            for t in range(2):
                w = wp.tile([128, 1], F32, name=f"wpos{t}", tag=f"wpos{t}")
                nc.sync.dma_start(w[:], pw[t])
                Wpos.append(w)

            def vec(name, src, p=128):
                v = wp.tile([p, 1], F32, name=name, tag=name)
                nc.sync.dma_start(v[:], src)
                return v

            Bhr = vec("bhr", hrb[:], p=64)
            Bce = vec("bce", ceb[:], p=25)
            Bpos = vec("bpos", pb[:], p=1)
            Sc = [[vec(f"sc{c}{t}", cs[c, t]) for t in range(2)] for c in range(2)]
            Bc = [[vec(f"bc{c}{t}", cb[c, t]) for t in range(2)] for c in range(2)]
            Sr3 = [[vec(f"sr3_{i}{t}", rs3[i, t]) for t in range(2)] for i in range(3)]
            Br3 = [[vec(f"br3_{i}{t}", rb3[i, t]) for t in range(2)] for i in range(3)]
            Sr1 = [[vec(f"sr1_{i}{t}", rs1[i, t]) for t in range(2)] for i in range(3)]
            Br1 = [[vec(f"br1_{i}{t}", rb1[i, t]) for t in range(2)] for i in range(3)]
            Ssup = [vec(f"ssup{t}", ss[t]) for t in range(2)]
            Bsup = [vec(f"bsup{t}", sbe[t]) for t in range(2)]

            # f32r constants must come from a rounding producer (DVE copy)
            ones_f = wp.tile([25, 128], F32, name="ones_f", tag="ones_f")
            nc.vector.memset(ones_f[:], 1.0)
            ones25 = wp.tile([25, 1], F32R, name="ones25", tag="ones25")
            nc.vector.tensor_copy(ones25[:], ones_f[:, 0:1])
            ones128 = wp.tile([1, 128], F32R, name="ones128", tag="ones128")
            nc.vector.tensor_copy(ones128[:], ones_f[0:1, :])
            guard = None
            if DEBUG_TAPS:
                guard = wp.tile([128, 16], F32, name="guard", tag="guard")
                nc.vector.memset(guard[:], 0.0)

            for s in range(SPC):
                _emit_sample(
                    nc, tc, s,
                    feat=feat, p2s=p2s,
                    o_d=o_d, pos_d=pos_d, pup_d=pup_d,
                    Whr=Whr, Wce=Wce, Wr1=Wr1, Wpos=Wpos,
                    Wc=Wc, Wr3=Wr3, Wsup=Wsup,
                    Bhr=Bhr, Bce=Bce, Bpos=Bpos, Sc=Sc, Bc=Bc,
                    Sr3=Sr3, Br3=Br3, Sr1=Sr1, Br1=Br1,
                    Ssup=Ssup, Bsup=Bsup, ones25=ones25, ones128=ones128,
                    featb=featb,
                    wbp=wbp, padp=padp, upool=upool, rot=rot, psp=psp,
                    dbg=dbg, guard=guard,
                )

    nc.compile()
    return nc


def _emit_sample(nc, tc, s, *, feat, p2s, o_d, pos_d, pup_d,
                 Whr, Wce, Wr1, Wpos, Wc, Wr3, Wsup, Bhr, Bce, Bpos, Sc, Bc,
                 Sr3, Br3, Sr1, Br1, Ssup, Bsup, ones25, ones128, featb,
                 wbp, padp, upool, rot, psp, dbg=None, guard=None):

    def dump(key, tiles, pad=98):
        if dbg is None:
            return
        for t in range(2):
            n = pad * pad
            nc.sync.dma_start(dbg[key][s, t, :, 0:n],
                              tiles[t][:, 0:pad, 0:pad])

    def pad_tile(name, p=128):
        t = padp.tile([p, 98, 98], BF16, name=f"{name}_s{s}", tag="pad")
        return t

    def zero_borders(t):
        nc.vector.memset(t[:, 0, :], 0.0)
        nc.vector.memset(t[:, 97, :], 0.0)
        nc.vector.memset(t[:, 1:97, 0], 0.0)
        nc.vector.memset(t[:, 1:97, 97], 0.0)

    gsl = lambda g: slice(g * NT, (g + 1) * NT)

    # ---------- Phase A: hr 1x1 conv (f32r) -> padded guide ----------
    Gpad = pad_tile("gpad", p=64)
    zero_borders(Gpad)
    for g in range(G):
        ps = psp.tile([64, GR, 96], F32, name="ps_hr", tag="ps")
        for t in range(2):
            fs = rot.tile([128, NT], BF16, name="fsl", tag="fsl", bufs=5)
            nc.sync.dma_start(fs[:], featb[s, t, :, gsl(g)])
            nc.tensor.matmul(
                ps[:], Whr[t][:], _r3(fs[:]),
                start=(t == 0), stop=(t == 1),
            )
        nc.scalar.activation(
            Gpad[:, 1 + GR * g:1 + GR * (g + 1), 1:97], ps[:],
            AFT.Identity, bias=Bhr[:],
        )

    if dbg is not None:
        nc.sync.dma_start(dbg["gpad"][s], Gpad[:])

    # ---------- Phase B+C: ce 3x3 conv -> exp -> carafe -> wh_in/U ----------
    # Software-pipelined one group ahead: the 9 ce matmuls of group g+1
    # are emitted before the carafe sums of group g so the PE fills the
    # exp->mul->reciprocal serial window instead of idling.
    U = [upool.tile([128, 97, 97], BF16, name=f"u{t}_s{s}", tag="u") for t in range(2)]

    def ce_mms(g):
        psE = psp.tile([25, GR, 96], F32, name="ps_ce", tag="ps")
        for o9 in range(9):
            ky, kx = divmod(o9, 3)
            nc.tensor.matmul(
                psE[:], Wce[:, o9, :],
                Gpad[:, GR * g + ky:GR * g + ky + GR, kx:kx + 96],
                start=(o9 == 0), stop=(o9 == 8),
            )
        return psE

    def pub_whin(pu, g):
        pub = psp.tile([128, GR, 96], F32, name="pub", tag="ps")
        nc.tensor.matmul(pub[:], ones128[:], pu[:], start=True, stop=True)
        for t in range(2):
            fs = rot.tile([128, NT], F32, name="fsl2", tag="fsl", bufs=5)
            nc.sync.dma_start(fs[:], feat[s, t, :, gsl(g)])
            nc.vector.tensor_mul(
                U[t][:, 1 + GR * g:1 + GR * (g + 1), 1:97],
                _r3(fs[:]), pub[:],
            )

    psE_cur = ce_mms(0)
    pu_prev = None
    for g in range(G):
        eg = rot.tile([25, GR, 96], F32R, name="eg", tag="eg", bufs=3)
        nc.scalar.activation(eg[:], psE_cur[:], AFT.Exp, bias=Bce[:])
        vg = rot.tile([25, GR, 96], F32, name="vg", tag="vg", bufs=3)
        for ki in range(5):
            nc.sync.dma_start(vg[5 * ki:5 * ki + 5], _win_ap(p2s, s, g, ki))
        if dbg is not None:
            nc.sync.dma_start(dbg["v"][s, :, gsl(g)], vg[:])
            nc.sync.dma_start(dbg["e"][s, :, gsl(g)], eg[:])
        if g + 1 < G:
            psE_cur = ce_mms(g + 1)
        tg = rot.tile([25, GR, 96], F32R, name="tg", tag="tg", bufs=3)
        nc.vector.tensor_mul(tg[:], eg[:], vg[:])
        psN = psp.tile([1, GR, 96], F32, name="ps_n", tag="psn", bufs=2)
        nc.tensor.matmul(psN[:], ones25[:], tg[:], start=True, stop=True)
        psD = psp.tile([1, GR, 96], F32, name="ps_d", tag="psn", bufs=2)
        nc.tensor.matmul(psD[:], ones25[:], eg[:], start=True, stop=True)
        if pu_prev is not None:
            pub_whin(pu_prev, g - 1)
        rc = rot.tile([1, GR, 96], F32, name="rc", tag="sm", bufs=6)
        nc.vector.reciprocal(rc[:], psD[:])
        pu = rot.tile([1, GR, 96], F32R, name="pu", tag="sm", bufs=6)
        nc.vector.tensor_mul(pu[:], psN[:], rc[:])
        nc.gpsimd.dma_start(pup_d[s, gsl(g)], pu[:])
        pu_prev = pu
    pub_whin(pu_prev, G - 1)
    for t in range(2):
        if "uedge" in SKIP_OPS:
            continue
        nc.vector.tensor_copy(U[t][:, 0, 1:97], U[t][:, 1, 1:97])
        nc.vector.tensor_copy(U[t][:, :, 0], U[t][:, :, 1])
    dump("u", U, pad=97)

    # ---------- stencil conv helper (c1 / c2) ----------
    def cconv_mm(Wt, co_t, g, ps):
        k = 0
        for t in range(2):
            for d, (dy, dx) in enumerate(DELTAS):
                nc.tensor.matmul(
                    ps[:], Wt[t][:, d, co_t * 128:(co_t + 1) * 128],
                    U[t][:, GR * g + dy:GR * g + dy + GR, dx:dx + 96],
                    start=(k == 0), stop=(k == 7),
                )
                k += 1

    # ---------- Phase D: c1 -> P0 ----------
    Wc1 = Wc[0]
    if dbg is not None:
        for t in range(2):
            nc.sync.dma_start(dbg["wc1pre"][s, t, :, 0:4 * 256], Wc1[t][:])
    P0 = [pad_tile(f"p0_{t}") for t in range(2)]
    for t in range(2):
        zero_borders(P0[t])
    for co_t in range(2):
        for g in range(G):
            ps = psp.tile([128, GR, 96], F32, name="ps_c1", tag="ps")
            cconv_mm(Wc1, co_t, g, ps)
            nc.scalar.activation(
                P0[co_t][:, 1 + GR * g:1 + GR * (g + 1), 1:97], ps[:],
                AFT.Relu, bias=Bc[0][co_t][:], scale=Sc[0][co_t][:],
            )

    if dbg is not None:
        for t in range(2):
            nc.sync.dma_start(dbg["wc1"][s, t, :, 0:4 * 256], Wc1[t][:])
        nc.sync.dma_start(dbg["guard"][s], guard[:])
    dump("x1", P0)
    if MAX_PHASE <= 4:
        return

    # ---------- Phase E: RepVgg blocks ----------
    cur = P0
    for i in range(3):
        if MAX_PHASE <= 4 + i:
            break
        W3 = Wr3[i]
        nxt = [pad_tile(f"p{i + 1}_{t}") for t in range(2)]
        for t in range(2):
            zero_borders(nxt[t])
        for co_t in range(2):
            co = slice(co_t * 128, (co_t + 1) * 128)
            for g in range(G):
                ps3 = psp.tile([128, GR, 96], F32, name="ps_r3", tag="ps")
                k = 0
                for t in range(2):
                    for o9 in range(9):
                        ky, kx = divmod(o9, 3)
                        nc.tensor.matmul(
                            ps3[:], W3[t][:, o9, co],
                            cur[t][:, GR * g + ky:GR * g + ky + GR, kx:kx + 96],
                            start=(k == 0), stop=(k == 17),
                        )
                        k += 1
                ps1 = psp.tile([128, GR, 96], F32, name="ps_r1", tag="ps")
                for t in range(2):
                    nc.tensor.matmul(
                        ps1[:], Wr1[i][t][:, co],
                        cur[t][:, 1 + GR * g:1 + GR * (g + 1), 1:97],
                        start=(t == 0), stop=(t == 1),
                    )
                b3 = rot.tile([128, GR, 96], F32, name="b3", tag="b3")
                nc.scalar.activation(b3[:], ps3[:], AFT.Relu,
                                     bias=Br3[i][co_t][:], scale=Sr3[i][co_t][:])
                b1 = rot.tile([128, GR, 96], F32, name="b1", tag="b1")
                nc.scalar.activation(b1[:], ps1[:], AFT.Relu,
                                     bias=Br1[i][co_t][:], scale=Sr1[i][co_t][:])
                nc.vector.tensor_add(
                    nxt[co_t][:, 1 + GR * g:1 + GR * (g + 1), 1:97], b3[:], b1[:])
        cur = nxt
    dump("xr", cur)
    if MAX_PHASE <= 7:
        return

    # ---------- Phase F: c2 + merge -> WH ----------
    Wc2 = Wc[1]
    WH = [pad_tile(f"wh_{t}") for t in range(2)]
    for t in range(2):
        zero_borders(WH[t])
    for co_t in range(2):
        for g in range(G):
            ps = psp.tile([128, GR, 96], F32, name="ps_c2", tag="ps")
            cconv_mm(Wc2, co_t, g, ps)
            x2t = rot.tile([128, GR, 96], F32, name="x2t", tag="b3")
            nc.scalar.activation(x2t[:], ps[:], AFT.Relu,
                                 bias=Bc[1][co_t][:], scale=Sc[1][co_t][:])
            t1 = rot.tile([128, GR, 96], F32, name="t1", tag="b1")
            nc.vector.tensor_add(
                t1[:], x2t[:], cur[co_t][:, 1 + GR * g:1 + GR * (g + 1), 1:97])
            nc.vector.tensor_add(
                WH[co_t][:, 1 + GR * g:1 + GR * (g + 1), 1:97], t1[:],
                U[co_t][:, 1 + GR * g:1 + GR * (g + 1), 1:97])

    dump("wh", WH)
    if MAX_PHASE <= 8:
        return

    # ---------- Phase G+H: sup 3x3 + residual + pos 1x1 ----------
    for g in range(G):
        ots = []
        for co_t in range(2):
            co = slice(co_t * 128, (co_t + 1) * 128)
            ps = psp.tile([128, GR, 96], F32, name="ps_sup", tag="ps")
            k = 0
            for t in range(2):
                for o9 in range(9):
                    ky, kx = divmod(o9, 3)
                    nc.tensor.matmul(
                        ps[:], Wsup[t][:, o9, co],
                        WH[t][:, GR * g + ky:GR * g + ky + GR, kx:kx + 96],
                        start=(k == 0), stop=(k == 17),
                    )
                    k += 1
            st = rot.tile([128, GR, 96], F32, name="st", tag="b3")
            nc.scalar.activation(st[:], ps[:], AFT.Relu,
                                 bias=Bsup[co_t][:], scale=Ssup[co_t][:])
            fs = rot.tile([128, NT], F32, name="fsl3", tag="fsl", bufs=5)
            nc.sync.dma_start(fs[:], feat[s, co_t, :, gsl(g)])
            ot = rot.tile([128, GR, 96], F32, name="ot", tag="ot", bufs=3)
            nc.vector.tensor_add(ot[:], st[:], _r3(fs[:]))
            nc.gpsimd.dma_start(o_d[s, co_t, :, gsl(g)], ot[:])
            ots.append(ot)
        psP = psp.tile([1, GR, 96], F32, name="ps_pos", tag="psn", bufs=2)
        for co_t in range(2):
            nc.tensor.matmul(psP[:], Wpos[co_t][:], ots[co_t][:],
                             start=(co_t == 0), stop=(co_t == 1))
        pt = rot.tile([1, GR, 96], F32, name="pt", tag="sm", bufs=6)
        nc.scalar.activation(pt[:], psP[:], AFT.Identity, bias=Bpos[:])
        nc.gpsimd.dma_start(pos_d[s, gsl(g)], pt[:])


# ============================================================
# host side
# ============================================================

def _prep_weights(inputs):
    bf = ml_dtypes.bfloat16
    f = lambda x: np.ascontiguousarray(np.asarray(x, np.float32))
    w = {}
    w["hrw"] = np.ascontiguousarray(
        np.asarray(inputs["hrW"], np.float32)[:, :, 0, 0].T.reshape(2, 128, 64).astype(bf))
    w["hrb"] = f(inputs["hrB"]).reshape(64, 1)
    w["cew"] = np.ascontiguousarray(
        np.asarray(inputs["ceW"], np.float32).transpose(1, 2, 3, 0).reshape(64, 9, 25).astype(bf))
    w["ceb"] = f(inputs["ceB"]).reshape(25, 1)

    s = np.float32(0.5)
    cws = []
    for key in ("c1W", "c2W"):
        cW = f(inputs[key])[:, :, 0, 0]  # (256, 768)
        W_lh, W_hl, W_hh = cW[:, :256], cW[:, 256:512], cW[:, 512:768]
        Wd = np.stack([
            s * (W_lh + W_hl + W_hh),
            s * (-W_lh + W_hl - W_hh),
            s * (W_lh - W_hl - W_hh),
            s * (-W_lh - W_hl + W_hh),
        ])  # (4, co, ci)
        # -> [ci_t, 128ci, 4d, 256co]
        cws.append(Wd.transpose(2, 0, 1).reshape(2, 128, 4, 256))
    w["cw"] = np.ascontiguousarray(np.stack(cws).astype(bf))
    w["cs"] = np.stack([f(inputs["c1s"]), f(inputs["c2s"])]).reshape(2, 2, 128, 1)
    w["cb"] = np.stack([f(inputs["c1b"]), f(inputs["c2b"])]).reshape(2, 2, 128, 1)

    # repW3: (3, co, ci, ky, kx) -> [i, ci_t, 128ci, 9off, 256co]
    r3 = f(inputs["repW3"]).transpose(0, 2, 3, 4, 1).reshape(3, 2, 128, 9, 256)
    w["rw3"] = np.ascontiguousarray(r3.astype(bf))
    w["rs3"] = f(inputs["repS3"]).reshape(3, 2, 128, 1)
    w["rb3"] = f(inputs["repB3"]).reshape(3, 2, 128, 1)
    r1 = f(inputs["repW1"])[:, :, :, 0, 0].transpose(0, 2, 1).reshape(3, 2, 128, 256)
    w["rw1"] = np.ascontiguousarray(r1.astype(bf))
    w["rs1"] = f(inputs["repS1"]).reshape(3, 2, 128, 1)
    w["rb1"] = f(inputs["repB1"]).reshape(3, 2, 128, 1)

    sW = f(inputs["supW"]).transpose(1, 2, 3, 0).reshape(2, 128, 9, 256)
    w["sw"] = np.ascontiguousarray(sW.astype(bf))
    w["ss"] = f(inputs["supS"]).reshape(2, 128, 1)
    w["sbe"] = (f(inputs["supCb"]) * f(inputs["supS"]) + f(inputs["supB"])).reshape(2, 128, 1)

    w["pw"] = f(inputs["posW"])[0, :, 0, 0].reshape(2, 128, 1)
    w["pb"] = f(inputs["posB"]).reshape(1, 1)
    return w


def kernel(**inputs):
    nc = _CACHE.get("nc")
    if nc is None:
        nc = _build_program()
        _CACHE["nc"] = nc

    feat = np.ascontiguousarray(np.asarray(inputs["feat"], np.float32))
    featb = np.ascontiguousarray(feat.astype(ml_dtypes.bfloat16))
    pred = np.asarray(inputs["pred"], np.float32)
    B = feat.shape[0]

    xp = np.pad(pred[:, 0], ((0, 0), (2, 2), (2, 2)), mode="reflect")
    p2 = np.ascontiguousarray(xp.repeat(2, axis=1).repeat(2, axis=2))  # (B,104,104)

    w = _prep_weights(inputs)
    in_maps = []
    for c in range(NCORES):
        m = dict(w)
        m["feat"] = feat[SPC * c:SPC * (c + 1)].reshape(SPC, 2, 128, PIX)
        m["featb"] = featb[SPC * c:SPC * (c + 1)].reshape(SPC, 2, 128, PIX)
        m["p2s"] = p2[SPC * c:SPC * (c + 1)]
        in_maps.append(m)

    res = run_bass_kernel_spmd(nc, in_maps, core_ids=list(range(NCORES)))
    outs = res.results

    out = np.concatenate([r["o"].reshape(SPC, 256, 96, 96) for r in outs])
    pos = np.concatenate([r["pos"].reshape(SPC, 1, 96, 96) for r in outs])
    pup = np.concatenate([r["pup"].reshape(SPC, 1, 96, 96) for r in outs])
    return out, pos, pup


# revision 75
# speedup vs baseline: 1.0358x; 1.0250x over previous
"""Distributed Trainium2 Bass kernel for the CARAFE+SWT+CSPRep block.

Strategy: pure data parallel — 16 samples sharded 2-per-core across 8
NeuronCores; all weights replicated. The whole per-sample network runs
on-chip in one NEFF:

  hr 1x1 conv (f32r) -> ce 3x3 conv (bf16) -> exp -> carafe (window DMA
  + partition-sum matmuls + reciprocal) -> wh_in -> [SWT+concat+1x1
  folded into a 2x2 stencil conv] c1/c2 -> 3x RepVgg (3x3+1x1, bf16)
  -> sup 3x3 -> +feat residual -> pos 1x1 (f32r).

Key rewrites (validated against the reference in numpy):
  * The stationary Haar bands (lh,hl,hh) are linear in 2x2 shifted
    copies of wh_in, so conv1x1(concat(bands)) == conv2x2(wh_in) with
    host-transformed weights (clamped edge handled by a 97x97 padded
    buffer with replicated top row / left col).
  * CARAFE softmax-normalize + weighted sum == (sum_k e_k*v_k)/(sum_k
    e_k) with e = exp(raw); the k=25 shifted upsampled views of pred
    are materialized by a single strided-window DMA from a host-built
    reflect-padded 2x-upsampled pred plane (pure indexing, no math).
  * BN scale/shift fused into the PSUM->SBUF evacuation on ScalarE
    (out = Relu(psum*scale + bias)); relu(relu(a)+relu(b)) == the sum.
"""

import numpy as np
import ml_dtypes

import bass_rust
import concourse.bass as bass
import concourse.tile as tile
from concourse import bacc, mybir
from concourse.bass_utils import run_bass_kernel_spmd

F32 = mybir.dt.float32
F32R = mybir.dt.float32r
BF16 = mybir.dt.bfloat16
AFT = mybir.ActivationFunctionType

NCORES = 8
SPC = 2            # samples per core
PIX = 96 * 96      # 9216
GR = 4             # output rows per matmul tile
G = 96 // GR       # 24 row groups
NT = GR * 96       # 384 free elems per matmul
DELTAS = [(0, 0), (0, 1), (1, 0), (1, 1)]

_CACHE = {}
DEBUG_TAPS = False  # emit extra dram outputs of intermediates
MAX_PHASE = 99      # debug: limit emission (4=c1, 5=rep0, 6=rep1, 7=rep2, 8=c2, 9=sup)
REC_OFF = False     # debug: replace reciprocal with copy
SKIP_OPS = ()       # debug: skip named ops ("exp","vdma","tg","sums","pub","whin","uedge")


def _r3(ap, a=GR):
    """[P, a*b] -> [P, a, b] view."""
    return ap.rearrange("p (a b) -> p a b", a=a)


def _win_ap(p2s_ap, s, g, ki):
    """Strided window view of the upsampled padded pred plane: CARAFE
    taps (ki, 0..4) for output rows [4g, 4g+4) as a [5,4,96] DMA source."""
    w = p2s_ap.copy()
    w.ap = bass_rust.VecI64Pair([[2, 5], [104, GR], [1, 96]])
    w.offset = s * 104 * 104 + GR * g * 104 + 2 * 104 * ki
    return w


def _build_program():
    nc = bacc.Bacc(
        "TRN2",
        target_bir_lowering=False,
        debug=False,
        enable_asserts=False,
        num_devices=NCORES,
    )

    def din(name, shape, dt=F32):
        return nc.dram_tensor(name, shape, dt, kind="ExternalInput").ap()

    def dout(name, shape, dt=F32):
        return nc.dram_tensor(name, shape, dt, kind="ExternalOutput").ap()

    feat = din("feat", [SPC, 2, 128, PIX])
    featb = din("featb", [SPC, 2, 128, PIX], BF16)
    p2s = din("p2s", [SPC, 104, 104])
    hrw = din("hrw", [2, 128, 64], BF16)
    hrb = din("hrb", [64, 1])
    cew = din("cew", [64, 9, 25], BF16)
    ceb = din("ceb", [25, 1])
    cw = din("cw", [2, 2, 128, 4, 256], BF16)    # conv, ci_t, ci, delta, co
    cs = din("cs", [2, 2, 128, 1])               # conv, co_t, co, 1
    cb = din("cb", [2, 2, 128, 1])
    rw3 = din("rw3", [3, 2, 128, 9, 256], BF16)
    rs3 = din("rs3", [3, 2, 128, 1])
    rb3 = din("rb3", [3, 2, 128, 1])
    rw1 = din("rw1", [3, 2, 128, 256], BF16)
    rs1 = din("rs1", [3, 2, 128, 1])
    rb1 = din("rb1", [3, 2, 128, 1])
    sw = din("sw", [2, 128, 9, 256], BF16)
    ss = din("ss", [2, 128, 1])
    sbe = din("sbe", [2, 128, 1])
    pw = din("pw", [2, 128, 1])
    pb = din("pb", [1, 1])

    o_d = dout("o", [SPC, 2, 128, PIX], F32R)
    pos_d = dout("pos", [SPC, PIX])
    pup_d = dout("pup", [SPC, PIX], F32R)
    dbg = None
    if DEBUG_TAPS:
        dbg = {k: dout(f"dbg_{k}", [SPC, 2, 128, 98 * 98], BF16)
               for k in ("u", "x1", "xr", "wh")}
        dbg["wc1"] = dout("dbg_wc1", [SPC, 2, 128, 9 * 256], BF16)
        dbg["wc1pre"] = dout("dbg_wc1pre", [SPC, 2, 128, 9 * 256], BF16)
        dbg["guard"] = dout("dbg_guard", [SPC, 128, 16], F32)
        dbg["gpad"] = dout("dbg_gpad", [SPC, 64, 98 * 98], BF16)
        dbg["v"] = dout("dbg_v", [SPC, 25, PIX], F32)
        dbg["e"] = dout("dbg_e", [SPC, 25, PIX], F32R)

    with tile.TileContext(nc) as tc:
        with (
            tc.tile_pool(name="wconst", bufs=1) as wp,
            tc.tile_pool(name="wbig", bufs=4) as wbp,
            tc.tile_pool(name="pad", bufs=4) as padp,
            tc.tile_pool(name="upool", bufs=2) as upool,
            tc.tile_pool(name="rot", bufs=3) as rot,
            tc.tile_pool(name="psp", bufs=6, space="PSUM") as psp,
        ):
            # ---- persistent small weights ----
            Whr = []
            for t in range(2):
                w = wp.tile([128, 64], BF16, name=f"whr{t}", tag=f"whr{t}")
                nc.sync.dma_start(w[:], hrw[t])
                Whr.append(w)
            Wce = wp.tile([64, 9, 25], BF16, name="wce", tag="wce")
            nc.sync.dma_start(Wce[:], cew[:])
            Wr1 = [[None] * 2 for _ in range(3)]
            for i in range(3):
                for t in range(2):
                    w = wp.tile([128, 256], BF16, name=f"wr1_{i}{t}", tag=f"wr1_{i}{t}")
                    nc.sync.dma_start(w[:], rw1[i, t])
                    Wr1[i][t] = w
            Wc = [[None] * 2 for _ in range(2)]
            for cv in range(2):
                for t in range(2):
                    w = wp.tile([128, 4, 256], BF16, name=f"wcv{cv}{t}", tag=f"wcv{cv}{t}")
                    nc.sync.dma_start(w[:], cw[cv, t])
                    Wc[cv][t] = w
            Wr3 = [[None] * 2 for _ in range(3)]
            for i in range(3):
                for t in range(2):
                    w = wp.tile([128, 9, 256], BF16, name=f"wr3_{i}{t}", tag=f"wr3_{i}{t}")
                    nc.scalar.dma_start(w[:], rw3[i, t])
                    Wr3[i][t] = w
            Wsup = []
            for t in range(2):
                w = wp.tile([128, 9, 256], BF16, name=f"wsup{t}", tag=f"wsup{t}")
                nc.scalar.dma_start(w[:], sw[t])
                Wsup.append(w)
            Wpos = []
            for t in range(2):
                wf = wp.tile([128, 1], F32, name=f"wposf{t}", tag=f"wposf{t}")
                nc.sync.dma_start(wf[:], pw[t])
                w = wp.tile([128, 1], F32R, name=f"wpos{t}", tag=f"wpos{t}")
                nc.vector.tensor_copy(w[:], wf[:])
                Wpos.append(w)

            def vec(name, src, p=128):
                v = wp.tile([p, 1], F32, name=name, tag=name)
                nc.sync.dma_start(v[:], src)
                return v

            Bhr = vec("bhr", hrb[:], p=64)
            Bce = vec("bce", ceb[:], p=25)
            Bpos = vec("bpos", pb[:], p=1)
            Sc = [[vec(f"sc{c}{t}", cs[c, t]) for t in range(2)] for c in range(2)]
            Bc = [[vec(f"bc{c}{t}", cb[c, t]) for t in range(2)] for c in range(2)]
            Sr3 = [[vec(f"sr3_{i}{t}", rs3[i, t]) for t in range(2)] for i in range(3)]
            Br3 = [[vec(f"br3_{i}{t}", rb3[i, t]) for t in range(2)] for i in range(3)]
            Sr1 = [[vec(f"sr1_{i}{t}", rs1[i, t]) for t in range(2)] for i in range(3)]
            Br1 = [[vec(f"br1_{i}{t}", rb1[i, t]) for t in range(2)] for i in range(3)]
            Ssup = [vec(f"ssup{t}", ss[t]) for t in range(2)]
            Bsup = [vec(f"bsup{t}", sbe[t]) for t in range(2)]

            # f32r constants must come from a rounding producer (DVE copy)
            ones_f = wp.tile([25, 128], F32, name="ones_f", tag="ones_f")
            nc.vector.memset(ones_f[:], 1.0)
            ones25 = wp.tile([25, 1], F32R, name="ones25", tag="ones25")
            nc.vector.tensor_copy(ones25[:], ones_f[:, 0:1])
            ones128 = wp.tile([1, 128], F32R, name="ones128", tag="ones128")
            nc.vector.tensor_copy(ones128[:], ones_f[0:1, :])
            guard = None
            if DEBUG_TAPS:
                guard = wp.tile([128, 16], F32, name="guard", tag="guard")
                nc.vector.memset(guard[:], 0.0)

            for s in range(SPC):
                _emit_sample(
                    nc, tc, s,
                    feat=feat, p2s=p2s,
                    o_d=o_d, pos_d=pos_d, pup_d=pup_d,
                    Whr=Whr, Wce=Wce, Wr1=Wr1, Wpos=Wpos,
                    Wc=Wc, Wr3=Wr3, Wsup=Wsup,
                    Bhr=Bhr, Bce=Bce, Bpos=Bpos, Sc=Sc, Bc=Bc,
                    Sr3=Sr3, Br3=Br3, Sr1=Sr1, Br1=Br1,
                    Ssup=Ssup, Bsup=Bsup, ones25=ones25, ones128=ones128,
                    featb=featb,
                    wbp=wbp, padp=padp, upool=upool, rot=rot, psp=psp,
                    dbg=dbg, guard=guard,
                )

    nc.compile()
    return nc


def _emit_sample(nc, tc, s, *, feat, p2s, o_d, pos_d, pup_d,
                 Whr, Wce, Wr1, Wpos, Wc, Wr3, Wsup, Bhr, Bce, Bpos, Sc, Bc,
                 Sr3, Br3, Sr1, Br1, Ssup, Bsup, ones25, ones128, featb,
                 wbp, padp, upool, rot, psp, dbg=None, guard=None):

    def dump(key, tiles, pad=98):
        if dbg is None:
            return
        for t in range(2):
            n = pad * pad
            nc.sync.dma_start(dbg[key][s, t, :, 0:n],
                              tiles[t][:, 0:pad, 0:pad])

    def pad_tile(name, p=128):
        t = padp.tile([p, 98, 98], BF16, name=f"{name}_s{s}", tag="pad")
        return t

    def zero_borders(t):
        nc.vector.memset(t[:, 0, :], 0.0)
        nc.vector.memset(t[:, 97, :], 0.0)
        nc.vector.memset(t[:, 1:97, 0], 0.0)
        nc.vector.memset(t[:, 1:97, 97], 0.0)

    gsl = lambda g: slice(g * NT, (g + 1) * NT)

    # ---------- Phase A: hr 1x1 conv (f32r) -> padded guide ----------
    Gpad = pad_tile("gpad", p=64)
    zero_borders(Gpad)
    for g in range(G):
        ps = psp.tile([64, GR, 96], F32, name="ps_hr", tag="ps")
        for t in range(2):
            fs = rot.tile([128, NT], BF16, name="fsl", tag="fsl", bufs=5)
            nc.sync.dma_start(fs[:], featb[s, t, :, gsl(g)])
            nc.tensor.matmul(
                ps[:], Whr[t][:], _r3(fs[:]),
                start=(t == 0), stop=(t == 1),
            )
        nc.scalar.activation(
            Gpad[:, 1 + GR * g:1 + GR * (g + 1), 1:97], ps[:],
            AFT.Identity, bias=Bhr[:],
        )

    if dbg is not None:
        nc.sync.dma_start(dbg["gpad"][s], Gpad[:])

    # ---------- Phase B+C: ce 3x3 conv -> exp -> carafe -> wh_in/U ----------
    # Software-pipelined one group ahead: the 9 ce matmuls of group g+1
    # are emitted before the carafe sums of group g so the PE fills the
    # exp->mul->reciprocal serial window instead of idling.
    U = [upool.tile([128, 97, 97], BF16, name=f"u{t}_s{s}", tag="u") for t in range(2)]

    def ce_mms(g):
        psE = psp.tile([25, GR, 96], F32, name="ps_ce", tag="ps")
        for o9 in range(9):
            ky, kx = divmod(o9, 3)
            nc.tensor.matmul(
                psE[:], Wce[:, o9, :],
                Gpad[:, GR * g + ky:GR * g + ky + GR, kx:kx + 96],
                start=(o9 == 0), stop=(o9 == 8),
            )
        return psE

    def pub_whin(pu, g):
        pub = psp.tile([128, GR, 96], F32, name="pub", tag="ps")
        nc.tensor.matmul(pub[:], ones128[:], pu[:], start=True, stop=True)
        for t in range(2):
            fs = rot.tile([128, NT], F32, name="fsl2", tag="fsl", bufs=5)
            nc.sync.dma_start(fs[:], feat[s, t, :, gsl(g)])
            nc.vector.tensor_mul(
                U[t][:, 1 + GR * g:1 + GR * (g + 1), 1:97],
                _r3(fs[:]), pub[:],
            )

    psE_cur = ce_mms(0)
    pu_prev = None
    for g in range(G):
        eg = rot.tile([25, GR, 96], F32R, name="eg", tag="eg", bufs=3)
        nc.scalar.activation(eg[:], psE_cur[:], AFT.Exp, bias=Bce[:])
        vg = rot.tile([25, GR, 96], F32, name="vg", tag="vg", bufs=3)
        for ki in range(5):
            nc.sync.dma_start(vg[5 * ki:5 * ki + 5], _win_ap(p2s, s, g, ki))
        if dbg is not None:
            nc.sync.dma_start(dbg["v"][s, :, gsl(g)], vg[:])
            nc.sync.dma_start(dbg["e"][s, :, gsl(g)], eg[:])
        if g + 1 < G:
            psE_cur = ce_mms(g + 1)
        tg = rot.tile([25, GR, 96], F32R, name="tg", tag="tg", bufs=3)
        nc.vector.tensor_mul(tg[:], eg[:], vg[:])
        psN = psp.tile([1, GR, 96], F32, name="ps_n", tag="psn", bufs=2)
        nc.tensor.matmul(psN[:], ones25[:], tg[:], start=True, stop=True)
        psD = psp.tile([1, GR, 96], F32, name="ps_d", tag="psn", bufs=2)
        nc.tensor.matmul(psD[:], ones25[:], eg[:], start=True, stop=True)
        if pu_prev is not None:
            pub_whin(pu_prev, g - 1)
        rc = rot.tile([1, GR, 96], F32, name="rc", tag="sm", bufs=6)
        nc.vector.reciprocal(rc[:], psD[:])
        pu = rot.tile([1, GR, 96], F32R, name="pu", tag="sm", bufs=6)
        nc.vector.tensor_mul(pu[:], psN[:], rc[:])
        nc.gpsimd.dma_start(pup_d[s, gsl(g)], pu[:])
        pu_prev = pu
    pub_whin(pu_prev, G - 1)
    for t in range(2):
        if "uedge" in SKIP_OPS:
            continue
        nc.vector.tensor_copy(U[t][:, 0, 1:97], U[t][:, 1, 1:97])
        nc.vector.tensor_copy(U[t][:, :, 0], U[t][:, :, 1])
    dump("u", U, pad=97)

    # ---------- stencil conv helper (c1 / c2) ----------
    def cconv_mm(Wt, co_t, g, ps):
        k = 0
        for t in range(2):
            for d, (dy, dx) in enumerate(DELTAS):
                nc.tensor.matmul(
                    ps[:], Wt[t][:, d, co_t * 128:(co_t + 1) * 128],
                    U[t][:, GR * g + dy:GR * g + dy + GR, dx:dx + 96],
                    start=(k == 0), stop=(k == 7),
                )
                k += 1

    # ---------- Phase D: c1 -> P0 ----------
    Wc1 = Wc[0]
    if dbg is not None:
        for t in range(2):
            nc.sync.dma_start(dbg["wc1pre"][s, t, :, 0:4 * 256], Wc1[t][:])
    P0 = [pad_tile(f"p0_{t}") for t in range(2)]
    for t in range(2):
        zero_borders(P0[t])
    for co_t in range(2):
        for g in range(G):
            ps = psp.tile([128, GR, 96], F32, name="ps_c1", tag="ps")
            cconv_mm(Wc1, co_t, g, ps)
            nc.scalar.activation(
                P0[co_t][:, 1 + GR * g:1 + GR * (g + 1), 1:97], ps[:],
                AFT.Relu, bias=Bc[0][co_t][:], scale=Sc[0][co_t][:],
            )

    if dbg is not None:
        for t in range(2):
            nc.sync.dma_start(dbg["wc1"][s, t, :, 0:4 * 256], Wc1[t][:])
        nc.sync.dma_start(dbg["guard"][s], guard[:])
    dump("x1", P0)
    if MAX_PHASE <= 4:
        return

    # ---------- Phase E: RepVgg blocks ----------
    cur = P0
    for i in range(3):
        if MAX_PHASE <= 4 + i:
            break
        W3 = Wr3[i]
        nxt = [pad_tile(f"p{i + 1}_{t}") for t in range(2)]
        for t in range(2):
            zero_borders(nxt[t])
        for co_t in range(2):
            co = slice(co_t * 128, (co_t + 1) * 128)
            for g in range(G):
                ps3 = psp.tile([128, GR, 96], F32, name="ps_r3", tag="ps")
                k = 0
                for t in range(2):
                    for o9 in range(9):
                        ky, kx = divmod(o9, 3)
                        nc.tensor.matmul(
                            ps3[:], W3[t][:, o9, co],
                            cur[t][:, GR * g + ky:GR * g + ky + GR, kx:kx + 96],
                            start=(k == 0), stop=(k == 17),
                        )
                        k += 1
                ps1 = psp.tile([128, GR, 96], F32, name="ps_r1", tag="ps")
                for t in range(2):
                    nc.tensor.matmul(
                        ps1[:], Wr1[i][t][:, co],
                        cur[t][:, 1 + GR * g:1 + GR * (g + 1), 1:97],
                        start=(t == 0), stop=(t == 1),
                    )
                b3 = rot.tile([128, GR, 96], F32, name="b3", tag="b3")
                nc.scalar.activation(b3[:], ps3[:], AFT.Relu,
                                     bias=Br3[i][co_t][:], scale=Sr3[i][co_t][:])
                b1 = rot.tile([128, GR, 96], F32, name="b1", tag="b1")
                nc.scalar.activation(b1[:], ps1[:], AFT.Relu,
                                     bias=Br1[i][co_t][:], scale=Sr1[i][co_t][:])
                nc.vector.tensor_add(
                    nxt[co_t][:, 1 + GR * g:1 + GR * (g + 1), 1:97], b3[:], b1[:])
        cur = nxt
    dump("xr", cur)
    if MAX_PHASE <= 7:
        return

    # ---------- Phase F: c2 + merge -> WH ----------
    Wc2 = Wc[1]
    WH = [pad_tile(f"wh_{t}") for t in range(2)]
    for t in range(2):
        zero_borders(WH[t])
    for co_t in range(2):
        for g in range(G):
            ps = psp.tile([128, GR, 96], F32, name="ps_c2", tag="ps")
            cconv_mm(Wc2, co_t, g, ps)
            x2t = rot.tile([128, GR, 96], F32, name="x2t", tag="b3")
            nc.scalar.activation(x2t[:], ps[:], AFT.Relu,
                                 bias=Bc[1][co_t][:], scale=Sc[1][co_t][:])
            t1 = rot.tile([128, GR, 96], F32, name="t1", tag="b1")
            nc.vector.tensor_add(
                t1[:], x2t[:], cur[co_t][:, 1 + GR * g:1 + GR * (g + 1), 1:97])
            nc.vector.tensor_add(
                WH[co_t][:, 1 + GR * g:1 + GR * (g + 1), 1:97], t1[:],
                U[co_t][:, 1 + GR * g:1 + GR * (g + 1), 1:97])

    dump("wh", WH)
    if MAX_PHASE <= 8:
        return

    # ---------- Phase G+H: sup 3x3 + residual + pos 1x1 ----------
    for g in range(G):
        ots = []
        for co_t in range(2):
            co = slice(co_t * 128, (co_t + 1) * 128)
            ps = psp.tile([128, GR, 96], F32, name="ps_sup", tag="ps")
            k = 0
            for t in range(2):
                for o9 in range(9):
                    ky, kx = divmod(o9, 3)
                    nc.tensor.matmul(
                        ps[:], Wsup[t][:, o9, co],
                        WH[t][:, GR * g + ky:GR * g + ky + GR, kx:kx + 96],
                        start=(k == 0), stop=(k == 17),
                    )
                    k += 1
            st = rot.tile([128, GR, 96], F32, name="st", tag="b3")
            nc.scalar.activation(st[:], ps[:], AFT.Relu,
                                 bias=Bsup[co_t][:], scale=Ssup[co_t][:])
            fs = rot.tile([128, NT], F32, name="fsl3", tag="fsl", bufs=5)
            nc.sync.dma_start(fs[:], feat[s, co_t, :, gsl(g)])
            ot = rot.tile([128, GR, 96], F32R, name="ot", tag="ot", bufs=3)
            nc.vector.tensor_add(ot[:], st[:], _r3(fs[:]))
            nc.gpsimd.dma_start(o_d[s, co_t, :, gsl(g)], ot[:])
            ots.append(ot)
        psP = psp.tile([1, GR, 96], F32, name="ps_pos", tag="psn", bufs=2)
        for co_t in range(2):
            nc.tensor.matmul(psP[:], Wpos[co_t][:], ots[co_t][:],
                             start=(co_t == 0), stop=(co_t == 1))
        pt = rot.tile([1, GR, 96], F32, name="pt", tag="sm", bufs=6)
        nc.scalar.activation(pt[:], psP[:], AFT.Identity, bias=Bpos[:])
        nc.gpsimd.dma_start(pos_d[s, gsl(g)], pt[:])


# ============================================================
# host side
# ============================================================

def _prep_weights(inputs):
    bf = ml_dtypes.bfloat16
    f = lambda x: np.ascontiguousarray(np.asarray(x, np.float32))
    w = {}
    w["hrw"] = np.ascontiguousarray(
        np.asarray(inputs["hrW"], np.float32)[:, :, 0, 0].T.reshape(2, 128, 64).astype(bf))
    w["hrb"] = f(inputs["hrB"]).reshape(64, 1)
    w["cew"] = np.ascontiguousarray(
        np.asarray(inputs["ceW"], np.float32).transpose(1, 2, 3, 0).reshape(64, 9, 25).astype(bf))
    w["ceb"] = f(inputs["ceB"]).reshape(25, 1)

    s = np.float32(0.5)
    cws = []
    for key in ("c1W", "c2W"):
        cW = f(inputs[key])[:, :, 0, 0]  # (256, 768)
        W_lh, W_hl, W_hh = cW[:, :256], cW[:, 256:512], cW[:, 512:768]
        Wd = np.stack([
            s * (W_lh + W_hl + W_hh),
            s * (-W_lh + W_hl - W_hh),
            s * (W_lh - W_hl - W_hh),
            s * (-W_lh - W_hl + W_hh),
        ])  # (4, co, ci)
        # -> [ci_t, 128ci, 4d, 256co]
        cws.append(Wd.transpose(2, 0, 1).reshape(2, 128, 4, 256))
    w["cw"] = np.ascontiguousarray(np.stack(cws).astype(bf))
    w["cs"] = np.stack([f(inputs["c1s"]), f(inputs["c2s"])]).reshape(2, 2, 128, 1)
    w["cb"] = np.stack([f(inputs["c1b"]), f(inputs["c2b"])]).reshape(2, 2, 128, 1)

    # repW3: (3, co, ci, ky, kx) -> [i, ci_t, 128ci, 9off, 256co]
    r3 = f(inputs["repW3"]).transpose(0, 2, 3, 4, 1).reshape(3, 2, 128, 9, 256)
    w["rw3"] = np.ascontiguousarray(r3.astype(bf))
    w["rs3"] = f(inputs["repS3"]).reshape(3, 2, 128, 1)
    w["rb3"] = f(inputs["repB3"]).reshape(3, 2, 128, 1)
    r1 = f(inputs["repW1"])[:, :, :, 0, 0].transpose(0, 2, 1).reshape(3, 2, 128, 256)
    w["rw1"] = np.ascontiguousarray(r1.astype(bf))
    w["rs1"] = f(inputs["repS1"]).reshape(3, 2, 128, 1)
    w["rb1"] = f(inputs["repB1"]).reshape(3, 2, 128, 1)

    sW = f(inputs["supW"]).transpose(1, 2, 3, 0).reshape(2, 128, 9, 256)
    w["sw"] = np.ascontiguousarray(sW.astype(bf))
    w["ss"] = f(inputs["supS"]).reshape(2, 128, 1)
    w["sbe"] = (f(inputs["supCb"]) * f(inputs["supS"]) + f(inputs["supB"])).reshape(2, 128, 1)

    w["pw"] = f(inputs["posW"])[0, :, 0, 0].reshape(2, 128, 1)
    w["pb"] = f(inputs["posB"]).reshape(1, 1)
    return w


def kernel(**inputs):
    nc = _CACHE.get("nc")
    if nc is None:
        nc = _build_program()
        _CACHE["nc"] = nc

    feat = np.ascontiguousarray(np.asarray(inputs["feat"], np.float32))
    featb = np.ascontiguousarray(feat.astype(ml_dtypes.bfloat16))
    pred = np.asarray(inputs["pred"], np.float32)
    B = feat.shape[0]

    xp = np.pad(pred[:, 0], ((0, 0), (2, 2), (2, 2)), mode="reflect")
    p2 = np.ascontiguousarray(xp.repeat(2, axis=1).repeat(2, axis=2))  # (B,104,104)

    w = _prep_weights(inputs)
    in_maps = []
    for c in range(NCORES):
        m = dict(w)
        m["feat"] = feat[SPC * c:SPC * (c + 1)].reshape(SPC, 2, 128, PIX)
        m["featb"] = featb[SPC * c:SPC * (c + 1)].reshape(SPC, 2, 128, PIX)
        m["p2s"] = p2[SPC * c:SPC * (c + 1)]
        in_maps.append(m)

    res = run_bass_kernel_spmd(nc, in_maps, core_ids=list(range(NCORES)))
    outs = res.results

    out = np.concatenate([r["o"].reshape(SPC, 256, 96, 96) for r in outs])
    pos = np.concatenate([r["pos"].reshape(SPC, 1, 96, 96) for r in outs])
    pup = np.concatenate([r["pup"].reshape(SPC, 1, 96, 96) for r in outs])
    return out, pos, pup


# revision 78
# speedup vs baseline: 1.0487x; 1.0124x over previous
"""Distributed Trainium2 Bass kernel for the CARAFE+SWT+CSPRep block.

Strategy: pure data parallel — 16 samples sharded 2-per-core across 8
NeuronCores; all weights replicated. The whole per-sample network runs
on-chip in one NEFF:

  hr 1x1 conv (f32r) -> ce 3x3 conv (bf16) -> exp -> carafe (window DMA
  + partition-sum matmuls + reciprocal) -> wh_in -> [SWT+concat+1x1
  folded into a 2x2 stencil conv] c1/c2 -> 3x RepVgg (3x3+1x1, bf16)
  -> sup 3x3 -> +feat residual -> pos 1x1 (f32r).

Key rewrites (validated against the reference in numpy):
  * The stationary Haar bands (lh,hl,hh) are linear in 2x2 shifted
    copies of wh_in, so conv1x1(concat(bands)) == conv2x2(wh_in) with
    host-transformed weights (clamped edge handled by a 97x97 padded
    buffer with replicated top row / left col).
  * CARAFE softmax-normalize + weighted sum == (sum_k e_k*v_k)/(sum_k
    e_k) with e = exp(raw); the k=25 shifted upsampled views of pred
    are materialized by a single strided-window DMA from a host-built
    reflect-padded 2x-upsampled pred plane (pure indexing, no math).
  * BN scale/shift fused into the PSUM->SBUF evacuation on ScalarE
    (out = Relu(psum*scale + bias)); relu(relu(a)+relu(b)) == the sum.
"""

import numpy as np
import ml_dtypes

import bass_rust
import concourse.bass as bass
import concourse.tile as tile
from concourse import bacc, mybir
from concourse.bass_utils import run_bass_kernel_spmd

F32 = mybir.dt.float32
F32R = mybir.dt.float32r
BF16 = mybir.dt.bfloat16
AFT = mybir.ActivationFunctionType

NCORES = 8
SPC = 2            # samples per core
PIX = 96 * 96      # 9216
GR = 4             # output rows per matmul tile
G = 96 // GR       # 24 row groups
NT = GR * 96       # 384 free elems per matmul
DELTAS = [(0, 0), (0, 1), (1, 0), (1, 1)]

_CACHE = {}
DEBUG_TAPS = False  # emit extra dram outputs of intermediates
MAX_PHASE = 99      # debug: limit emission (4=c1, 5=rep0, 6=rep1, 7=rep2, 8=c2, 9=sup)
REC_OFF = False     # debug: replace reciprocal with copy
SKIP_OPS = ()       # debug: skip named ops ("exp","vdma","tg","sums","pub","whin","uedge")


def _r3(ap, a=GR):
    """[P, a*b] -> [P, a, b] view."""
    return ap.rearrange("p (a b) -> p a b", a=a)


def _win_ap(p2s_ap, s, g, ki):
    """Strided window view of the upsampled padded pred plane: CARAFE
    taps (ki, 0..4) for output rows [4g, 4g+4) as a [5,4,96] DMA source."""
    w = p2s_ap.copy()
    w.ap = bass_rust.VecI64Pair([[2, 5], [104, GR], [1, 96]])
    w.offset = s * 104 * 104 + GR * g * 104 + 2 * 104 * ki
    return w


def _build_program():
    nc = bacc.Bacc(
        "TRN2",
        target_bir_lowering=False,
        debug=False,
        enable_asserts=False,
        num_devices=NCORES,
    )

    def din(name, shape, dt=F32):
        return nc.dram_tensor(name, shape, dt, kind="ExternalInput").ap()

    def dout(name, shape, dt=F32):
        return nc.dram_tensor(name, shape, dt, kind="ExternalOutput").ap()

    feat = din("feat", [SPC, 2, 128, PIX])
    featb = din("featb", [SPC, 2, 128, PIX], BF16)
    p2s = din("p2s", [SPC, 104, 104])
    hrw = din("hrw", [2, 128, 64], BF16)
    hrb = din("hrb", [64, 1])
    cew = din("cew", [64, 9, 25], BF16)
    ceb = din("ceb", [25, 1])
    cw = din("cw", [2, 2, 128, 4, 256], BF16)    # conv, ci_t, ci, delta, co
    cs = din("cs", [2, 2, 128, 1])               # conv, co_t, co, 1
    cb = din("cb", [2, 2, 128, 1])
    rw3 = din("rw3", [3, 2, 128, 9, 256], BF16)
    rs3 = din("rs3", [3, 2, 128, 1])
    rb3 = din("rb3", [3, 2, 128, 1])
    rw1 = din("rw1", [3, 2, 128, 256], BF16)
    rs1 = din("rs1", [3, 2, 128, 1])
    rb1 = din("rb1", [3, 2, 128, 1])
    sw = din("sw", [2, 128, 9, 256], BF16)
    ss = din("ss", [2, 128, 1])
    sbe = din("sbe", [2, 128, 1])
    pw = din("pw", [2, 128, 1])
    bpk = din("bpk", [128, 36])
    pb = din("pb", [1, 1])

    o_d = dout("o", [SPC, 2, 128, PIX], F32R)
    pos_d = dout("pos", [SPC, PIX])
    pup_d = dout("pup", [SPC, PIX], F32R)
    dbg = None
    if DEBUG_TAPS:
        dbg = {k: dout(f"dbg_{k}", [SPC, 2, 128, 98 * 98], BF16)
               for k in ("u", "x1", "xr", "wh")}
        dbg["wc1"] = dout("dbg_wc1", [SPC, 2, 128, 9 * 256], BF16)
        dbg["wc1pre"] = dout("dbg_wc1pre", [SPC, 2, 128, 9 * 256], BF16)
        dbg["guard"] = dout("dbg_guard", [SPC, 128, 16], F32)
        dbg["gpad"] = dout("dbg_gpad", [SPC, 64, 98 * 98], BF16)
        dbg["v"] = dout("dbg_v", [SPC, 25, PIX], F32)
        dbg["e"] = dout("dbg_e", [SPC, 25, PIX], F32R)

    with tile.TileContext(nc) as tc:
        with (
            tc.tile_pool(name="wconst", bufs=1) as wp,
            tc.tile_pool(name="wbig", bufs=4) as wbp,
            tc.tile_pool(name="pad", bufs=4) as padp,
            tc.tile_pool(name="upool", bufs=2) as upool,
            tc.tile_pool(name="rot", bufs=3) as rot,
            tc.tile_pool(name="psp", bufs=6, space="PSUM") as psp,
        ):
            # ---- persistent small weights ----
            Whr = []
            for t in range(2):
                w = wp.tile([128, 64], BF16, name=f"whr{t}", tag=f"whr{t}")
                nc.sync.dma_start(w[:], hrw[t])
                Whr.append(w)
            Wce = wp.tile([64, 9, 25], BF16, name="wce", tag="wce")
            nc.sync.dma_start(Wce[:], cew[:])
            Wr1 = [[None] * 2 for _ in range(3)]
            for i in range(3):
                for t in range(2):
                    w = wp.tile([128, 256], BF16, name=f"wr1_{i}{t}", tag=f"wr1_{i}{t}")
                    nc.sync.dma_start(w[:], rw1[i, t])
                    Wr1[i][t] = w
            Wc = [[None] * 2 for _ in range(2)]
            for cv in range(2):
                for t in range(2):
                    w = wp.tile([128, 4, 256], BF16, name=f"wcv{cv}{t}", tag=f"wcv{cv}{t}")
                    nc.sync.dma_start(w[:], cw[cv, t])
                    Wc[cv][t] = w
            Wr3 = [[None] * 2 for _ in range(3)]
            for i in range(3):
                for t in range(2):
                    w = wp.tile([128, 9, 256], BF16, name=f"wr3_{i}{t}", tag=f"wr3_{i}{t}")
                    nc.scalar.dma_start(w[:], rw3[i, t])
                    Wr3[i][t] = w
            Wsup = []
            for t in range(2):
                w = wp.tile([128, 9, 256], BF16, name=f"wsup{t}", tag=f"wsup{t}")
                nc.scalar.dma_start(w[:], sw[t])
                Wsup.append(w)
            Wpos = []
            for t in range(2):
                wf = wp.tile([128, 1], F32, name=f"wposf{t}", tag=f"wposf{t}")
                nc.sync.dma_start(wf[:], pw[t])
                w = wp.tile([128, 1], F32R, name=f"wpos{t}", tag=f"wpos{t}")
                nc.vector.tensor_copy(w[:], wf[:])
                Wpos.append(w)

            def vec(name, src, p=128):
                v = wp.tile([p, 1], F32, name=name, tag=name)
                nc.sync.dma_start(v[:], src)
                return v

            Bhr = vec("bhr", hrb[:], p=64)
            Bce = vec("bce", ceb[:], p=25)
            Bpos = vec("bpos", pb[:], p=1)
            Bpk = wp.tile([128, 36], F32, name="bpk_t", tag="bpk_t")
            nc.sync.dma_start(Bpk[:], bpk[:])
            col = lambda i: Bpk[:, i:i + 1]
            Sc = [[col(c * 2 + t) for t in range(2)] for c in range(2)]
            Bc = [[col(4 + c * 2 + t) for t in range(2)] for c in range(2)]
            Sr3 = [[col(8 + i * 2 + t) for t in range(2)] for i in range(3)]
            Br3 = [[col(14 + i * 2 + t) for t in range(2)] for i in range(3)]
            Sr1 = [[col(20 + i * 2 + t) for t in range(2)] for i in range(3)]
            Br1 = [[col(26 + i * 2 + t) for t in range(2)] for i in range(3)]
            Ssup = [col(32 + t) for t in range(2)]
            Bsup = [col(34 + t) for t in range(2)]

            # f32r constants must come from a rounding producer (DVE copy)
            ones_f = wp.tile([25, 128], F32, name="ones_f", tag="ones_f")
            nc.vector.memset(ones_f[:], 1.0)
            ones25 = wp.tile([25, 1], F32R, name="ones25", tag="ones25")
            nc.vector.tensor_copy(ones25[:], ones_f[:, 0:1])
            ones128 = wp.tile([1, 128], F32R, name="ones128", tag="ones128")
            nc.vector.tensor_copy(ones128[:], ones_f[0:1, :])
            guard = None
            if DEBUG_TAPS:
                guard = wp.tile([128, 16], F32, name="guard", tag="guard")
                nc.vector.memset(guard[:], 0.0)

            for s in range(SPC):
                _emit_sample(
                    nc, tc, s,
                    feat=feat, p2s=p2s,
                    o_d=o_d, pos_d=pos_d, pup_d=pup_d,
                    Whr=Whr, Wce=Wce, Wr1=Wr1, Wpos=Wpos,
                    Wc=Wc, Wr3=Wr3, Wsup=Wsup,
                    Bhr=Bhr, Bce=Bce, Bpos=Bpos, Sc=Sc, Bc=Bc,
                    Sr3=Sr3, Br3=Br3, Sr1=Sr1, Br1=Br1,
                    Ssup=Ssup, Bsup=Bsup, ones25=ones25, ones128=ones128,
                    featb=featb,
                    wbp=wbp, padp=padp, upool=upool, rot=rot, psp=psp,
                    dbg=dbg, guard=guard,
                )

    nc.compile()
    return nc


def _emit_sample(nc, tc, s, *, feat, p2s, o_d, pos_d, pup_d,
                 Whr, Wce, Wr1, Wpos, Wc, Wr3, Wsup, Bhr, Bce, Bpos, Sc, Bc,
                 Sr3, Br3, Sr1, Br1, Ssup, Bsup, ones25, ones128, featb,
                 wbp, padp, upool, rot, psp, dbg=None, guard=None):

    def dump(key, tiles, pad=98):
        if dbg is None:
            return
        for t in range(2):
            n = pad * pad
            nc.sync.dma_start(dbg[key][s, t, :, 0:n],
                              tiles[t][:, 0:pad, 0:pad])

    def pad_tile(name, p=128):
        t = padp.tile([p, 98, 98], BF16, name=f"{name}_s{s}", tag="pad")
        return t

    def zero_borders(t):
        nc.vector.memset(t[:, 0, :], 0.0)
        nc.vector.memset(t[:, 97, :], 0.0)
        nc.vector.memset(t[:, 1:97, 0], 0.0)
        nc.vector.memset(t[:, 1:97, 97], 0.0)

    gsl = lambda g: slice(g * NT, (g + 1) * NT)

    # ---------- Phase A: hr 1x1 conv (f32r) -> padded guide ----------
    Gpad = pad_tile("gpad", p=64)
    zero_borders(Gpad)
    for g in range(G):
        ps = psp.tile([64, GR, 96], F32, name="ps_hr", tag="ps")
        for t in range(2):
            fs = rot.tile([128, NT], BF16, name="fsl", tag="fsl", bufs=5)
            nc.sync.dma_start(fs[:], featb[s, t, :, gsl(g)])
            nc.tensor.matmul(
                ps[:], Whr[t][:], _r3(fs[:]),
                start=(t == 0), stop=(t == 1),
            )
        nc.scalar.activation(
            Gpad[:, 1 + GR * g:1 + GR * (g + 1), 1:97], ps[:],
            AFT.Identity, bias=Bhr[:],
        )

    if dbg is not None:
        nc.sync.dma_start(dbg["gpad"][s], Gpad[:])

    # ---------- Phase B+C: ce 3x3 conv -> exp -> carafe -> wh_in/U ----------
    # Software-pipelined one group ahead: the 9 ce matmuls of group g+1
    # are emitted before the carafe sums of group g so the PE fills the
    # exp->mul->reciprocal serial window instead of idling.
    U = [upool.tile([128, 97, 97], BF16, name=f"u{t}_s{s}", tag="u") for t in range(2)]

    def ce_mms(g):
        psE = psp.tile([25, GR, 96], F32, name="ps_ce", tag="ps")
        for o9 in range(9):
            ky, kx = divmod(o9, 3)
            nc.tensor.matmul(
                psE[:], Wce[:, o9, :],
                Gpad[:, GR * g + ky:GR * g + ky + GR, kx:kx + 96],
                start=(o9 == 0), stop=(o9 == 8),
            )
        return psE

    def pub_whin(pu, g):
        pub = psp.tile([128, GR, 96], F32, name="pub", tag="ps")
        nc.tensor.matmul(pub[:], ones128[:], pu[:], start=True, stop=True)
        for t in range(2):
            fs = rot.tile([128, NT], F32, name="fsl2", tag="fsl", bufs=5)
            nc.sync.dma_start(fs[:], feat[s, t, :, gsl(g)])
            nc.vector.tensor_mul(
                U[t][:, 1 + GR * g:1 + GR * (g + 1), 1:97],
                _r3(fs[:]), pub[:],
            )

    psE_cur = ce_mms(0)
    pu_prev = None
    for g in range(G):
        eg = rot.tile([25, GR, 96], F32R, name="eg", tag="eg", bufs=3)
        nc.scalar.activation(eg[:], psE_cur[:], AFT.Exp, bias=Bce[:])
        vg = rot.tile([25, GR, 96], F32, name="vg", tag="vg", bufs=3)
        for ki in range(5):
            nc.sync.dma_start(vg[5 * ki:5 * ki + 5], _win_ap(p2s, s, g, ki))
        if dbg is not None:
            nc.sync.dma_start(dbg["v"][s, :, gsl(g)], vg[:])
            nc.sync.dma_start(dbg["e"][s, :, gsl(g)], eg[:])
        if g + 1 < G:
            psE_cur = ce_mms(g + 1)
        tg = rot.tile([25, GR, 96], F32R, name="tg", tag="tg", bufs=3)
        nc.vector.tensor_mul(tg[:], eg[:], vg[:])
        psN = psp.tile([1, GR, 96], F32, name="ps_n", tag="psn", bufs=2)
        nc.tensor.matmul(psN[:], ones25[:], tg[:], start=True, stop=True)
        psD = psp.tile([1, GR, 96], F32, name="ps_d", tag="psn", bufs=2)
        nc.tensor.matmul(psD[:], ones25[:], eg[:], start=True, stop=True)
        if pu_prev is not None:
            pub_whin(pu_prev, g - 1)
        rc = rot.tile([1, GR, 96], F32, name="rc", tag="sm", bufs=6)
        nc.vector.reciprocal(rc[:], psD[:])
        pu = rot.tile([1, GR, 96], F32R, name="pu", tag="sm", bufs=6)
        nc.vector.tensor_mul(pu[:], psN[:], rc[:])
        nc.gpsimd.dma_start(pup_d[s, gsl(g)], pu[:])
        pu_prev = pu
    pub_whin(pu_prev, G - 1)
    for t in range(2):
        if "uedge" in SKIP_OPS:
            continue
        nc.vector.tensor_copy(U[t][:, 0, 1:97], U[t][:, 1, 1:97])
        nc.vector.tensor_copy(U[t][:, :, 0], U[t][:, :, 1])
    dump("u", U, pad=97)

    # ---------- stencil conv helper (c1 / c2) ----------
    def cconv_mm(Wt, co_t, g, ps):
        k = 0
        for t in range(2):
            for d, (dy, dx) in enumerate(DELTAS):
                nc.tensor.matmul(
                    ps[:], Wt[t][:, d, co_t * 128:(co_t + 1) * 128],
                    U[t][:, GR * g + dy:GR * g + dy + GR, dx:dx + 96],
                    start=(k == 0), stop=(k == 7),
                )
                k += 1

    # ---------- Phase D: c1 -> P0 ----------
    Wc1 = Wc[0]
    if dbg is not None:
        for t in range(2):
            nc.sync.dma_start(dbg["wc1pre"][s, t, :, 0:4 * 256], Wc1[t][:])
    P0 = [pad_tile(f"p0_{t}") for t in range(2)]
    for t in range(2):
        zero_borders(P0[t])
    for co_t in range(2):
        for g in range(G):
            ps = psp.tile([128, GR, 96], F32, name="ps_c1", tag="ps")
            cconv_mm(Wc1, co_t, g, ps)
            nc.scalar.activation(
                P0[co_t][:, 1 + GR * g:1 + GR * (g + 1), 1:97], ps[:],
                AFT.Relu, bias=Bc[0][co_t][:], scale=Sc[0][co_t][:],
            )

    if dbg is not None:
        for t in range(2):
            nc.sync.dma_start(dbg["wc1"][s, t, :, 0:4 * 256], Wc1[t][:])
        nc.sync.dma_start(dbg["guard"][s], guard[:])
    dump("x1", P0)
    if MAX_PHASE <= 4:
        return

    # ---------- Phase E: RepVgg blocks ----------
    cur = P0
    for i in range(3):
        if MAX_PHASE <= 4 + i:
            break
        W3 = Wr3[i]
        nxt = [pad_tile(f"p{i + 1}_{t}") for t in range(2)]
        for t in range(2):
            zero_borders(nxt[t])
        for co_t in range(2):
            co = slice(co_t * 128, (co_t + 1) * 128)
            for g in range(G):
                ps3 = psp.tile([128, GR, 96], F32, name="ps_r3", tag="ps")
                k = 0
                for t in range(2):
                    for o9 in range(9):
                        ky, kx = divmod(o9, 3)
                        nc.tensor.matmul(
                            ps3[:], W3[t][:, o9, co],
                            cur[t][:, GR * g + ky:GR * g + ky + GR, kx:kx + 96],
                            start=(k == 0), stop=(k == 17),
                        )
                        k += 1
                ps1 = psp.tile([128, GR, 96], F32, name="ps_r1", tag="ps")
                for t in range(2):
                    nc.tensor.matmul(
                        ps1[:], Wr1[i][t][:, co],
                        cur[t][:, 1 + GR * g:1 + GR * (g + 1), 1:97],
                        start=(t == 0), stop=(t == 1),
                    )
                b3 = rot.tile([128, GR, 96], F32, name="b3", tag="b3")
                nc.scalar.activation(b3[:], ps3[:], AFT.Relu,
                                     bias=Br3[i][co_t][:], scale=Sr3[i][co_t][:])
                b1 = rot.tile([128, GR, 96], F32, name="b1", tag="b1")
                nc.scalar.activation(b1[:], ps1[:], AFT.Relu,
                                     bias=Br1[i][co_t][:], scale=Sr1[i][co_t][:])
                nc.vector.tensor_add(
                    nxt[co_t][:, 1 + GR * g:1 + GR * (g + 1), 1:97], b3[:], b1[:])
        cur = nxt
    dump("xr", cur)
    if MAX_PHASE <= 7:
        return

    # ---------- Phase F: c2 + merge -> WH ----------
    Wc2 = Wc[1]
    WH = [pad_tile(f"wh_{t}") for t in range(2)]
    for t in range(2):
        zero_borders(WH[t])
    for co_t in range(2):
        for g in range(G):
            ps = psp.tile([128, GR, 96], F32, name="ps_c2", tag="ps")
            cconv_mm(Wc2, co_t, g, ps)
            x2t = rot.tile([128, GR, 96], F32, name="x2t", tag="b3")
            nc.scalar.activation(x2t[:], ps[:], AFT.Relu,
                                 bias=Bc[1][co_t][:], scale=Sc[1][co_t][:])
            t1 = rot.tile([128, GR, 96], F32, name="t1", tag="b1")
            nc.vector.tensor_add(
                t1[:], x2t[:], cur[co_t][:, 1 + GR * g:1 + GR * (g + 1), 1:97])
            nc.vector.tensor_add(
                WH[co_t][:, 1 + GR * g:1 + GR * (g + 1), 1:97], t1[:],
                U[co_t][:, 1 + GR * g:1 + GR * (g + 1), 1:97])

    dump("wh", WH)
    if MAX_PHASE <= 8:
        return

    # ---------- Phase G+H: sup 3x3 + residual + pos 1x1 ----------
    for g in range(G):
        ots = []
        for co_t in range(2):
            co = slice(co_t * 128, (co_t + 1) * 128)
            ps = psp.tile([128, GR, 96], F32, name="ps_sup", tag="ps")
            k = 0
            for t in range(2):
                for o9 in range(9):
                    ky, kx = divmod(o9, 3)
                    nc.tensor.matmul(
                        ps[:], Wsup[t][:, o9, co],
                        WH[t][:, GR * g + ky:GR * g + ky + GR, kx:kx + 96],
                        start=(k == 0), stop=(k == 17),
                    )
                    k += 1
            st = rot.tile([128, GR, 96], F32, name="st", tag="b3")
            nc.scalar.activation(st[:], ps[:], AFT.Relu,
                                 bias=Bsup[co_t][:], scale=Ssup[co_t][:])
            fs = rot.tile([128, NT], F32, name="fsl3", tag="fsl", bufs=5)
            nc.sync.dma_start(fs[:], feat[s, co_t, :, gsl(g)])
            ot = rot.tile([128, GR, 96], F32R, name="ot", tag="ot", bufs=3)
            nc.vector.tensor_add(ot[:], st[:], _r3(fs[:]))
            nc.gpsimd.dma_start(o_d[s, co_t, :, gsl(g)], ot[:])
            ots.append(ot)
        psP = psp.tile([1, GR, 96], F32, name="ps_pos", tag="psn", bufs=2)
        for co_t in range(2):
            nc.tensor.matmul(psP[:], Wpos[co_t][:], ots[co_t][:],
                             start=(co_t == 0), stop=(co_t == 1))
        pt = rot.tile([1, GR, 96], F32, name="pt", tag="sm", bufs=6)
        nc.scalar.activation(pt[:], psP[:], AFT.Identity, bias=Bpos[:])
        nc.gpsimd.dma_start(pos_d[s, gsl(g)], pt[:])


# ============================================================
# host side
# ============================================================

def _prep_weights(inputs):
    bf = ml_dtypes.bfloat16
    f = lambda x: np.ascontiguousarray(np.asarray(x, np.float32))
    w = {}
    w["hrw"] = np.ascontiguousarray(
        np.asarray(inputs["hrW"], np.float32)[:, :, 0, 0].T.reshape(2, 128, 64).astype(bf))
    w["hrb"] = f(inputs["hrB"]).reshape(64, 1)
    w["cew"] = np.ascontiguousarray(
        np.asarray(inputs["ceW"], np.float32).transpose(1, 2, 3, 0).reshape(64, 9, 25).astype(bf))
    w["ceb"] = f(inputs["ceB"]).reshape(25, 1)

    s = np.float32(0.5)
    cws = []
    for key in ("c1W", "c2W"):
        cW = f(inputs[key])[:, :, 0, 0]  # (256, 768)
        W_lh, W_hl, W_hh = cW[:, :256], cW[:, 256:512], cW[:, 512:768]
        Wd = np.stack([
            s * (W_lh + W_hl + W_hh),
            s * (-W_lh + W_hl - W_hh),
            s * (W_lh - W_hl - W_hh),
            s * (-W_lh - W_hl + W_hh),
        ])  # (4, co, ci)
        # -> [ci_t, 128ci, 4d, 256co]
        cws.append(Wd.transpose(2, 0, 1).reshape(2, 128, 4, 256))
    w["cw"] = np.ascontiguousarray(np.stack(cws).astype(bf))
    w["cs"] = np.stack([f(inputs["c1s"]), f(inputs["c2s"])]).reshape(2, 2, 128, 1)
    w["cb"] = np.stack([f(inputs["c1b"]), f(inputs["c2b"])]).reshape(2, 2, 128, 1)

    # repW3: (3, co, ci, ky, kx) -> [i, ci_t, 128ci, 9off, 256co]
    r3 = f(inputs["repW3"]).transpose(0, 2, 3, 4, 1).reshape(3, 2, 128, 9, 256)
    w["rw3"] = np.ascontiguousarray(r3.astype(bf))
    w["rs3"] = f(inputs["repS3"]).reshape(3, 2, 128, 1)
    w["rb3"] = f(inputs["repB3"]).reshape(3, 2, 128, 1)
    r1 = f(inputs["repW1"])[:, :, :, 0, 0].transpose(0, 2, 1).reshape(3, 2, 128, 256)
    w["rw1"] = np.ascontiguousarray(r1.astype(bf))
    w["rs1"] = f(inputs["repS1"]).reshape(3, 2, 128, 1)
    w["rb1"] = f(inputs["repB1"]).reshape(3, 2, 128, 1)

    sW = f(inputs["supW"]).transpose(1, 2, 3, 0).reshape(2, 128, 9, 256)
    w["sw"] = np.ascontiguousarray(sW.astype(bf))
    w["ss"] = f(inputs["supS"]).reshape(2, 128, 1)
    w["sbe"] = (f(inputs["supCb"]) * f(inputs["supS"]) + f(inputs["supB"])).reshape(2, 128, 1)

    w["pw"] = f(inputs["posW"])[0, :, 0, 0].reshape(2, 128, 1)
    w["bpk"] = np.ascontiguousarray(np.concatenate([
        w["cs"].reshape(4, 128), w["cb"].reshape(4, 128),
        w["rs3"].reshape(6, 128), w["rb3"].reshape(6, 128),
        w["rs1"].reshape(6, 128), w["rb1"].reshape(6, 128),
        w["ss"].reshape(2, 128), w["sbe"].reshape(2, 128),
    ]).T)
    w["pb"] = f(inputs["posB"]).reshape(1, 1)
    return w


def kernel(**inputs):
    nc = _CACHE.get("nc")
    if nc is None:
        nc = _build_program()
        _CACHE["nc"] = nc

    feat = np.ascontiguousarray(np.asarray(inputs["feat"], np.float32))
    featb = np.ascontiguousarray(feat.astype(ml_dtypes.bfloat16))
    pred = np.asarray(inputs["pred"], np.float32)
    B = feat.shape[0]

    xp = np.pad(pred[:, 0], ((0, 0), (2, 2), (2, 2)), mode="reflect")
    p2 = np.ascontiguousarray(xp.repeat(2, axis=1).repeat(2, axis=2))  # (B,104,104)

    w = _prep_weights(inputs)
    in_maps = []
    for c in range(NCORES):
        m = dict(w)
        m["feat"] = feat[SPC * c:SPC * (c + 1)].reshape(SPC, 2, 128, PIX)
        m["featb"] = featb[SPC * c:SPC * (c + 1)].reshape(SPC, 2, 128, PIX)
        m["p2s"] = p2[SPC * c:SPC * (c + 1)]
        in_maps.append(m)

    res = run_bass_kernel_spmd(nc, in_maps, core_ids=list(range(NCORES)))
    outs = res.results

    out = np.concatenate([r["o"].reshape(SPC, 256, 96, 96) for r in outs])
    pos = np.concatenate([r["pos"].reshape(SPC, 1, 96, 96) for r in outs])
    pup = np.concatenate([r["pup"].reshape(SPC, 1, 96, 96) for r in outs])
    return out, pos, pup
